# revision 17
# baseline (speedup 1.0000x reference)
"""Trainium2 Bass kernel for nn_NeighborModel (boundary refinement w/ sparse
neighborhood attention), SPMD over 8 NeuronCores.

Sharding: 2 groups x 4 cores; group g owns batch g; core q of a group owns a
256-channel chunk of that batch's feature maps (4 avg-pooled scales, cell-major
HWC layout, built on device). Each core gathers 7x7 neighborhoods around all
80 boundary points (7-cell runs via indirect DMA with flat element indices),
computes partial dot-products on DVE, then one AllGather per iteration
exchanges (qf chunk | partial dots). The transformer layer (80 tokens) is
tensor-parallel across the 4 cores of a group: core q owns a 320-column slice
of Q/K/V' (out_proj folded into V), a 512-column slice of lin1, and the
matching 512-row slice of lin2. All weights are SBUF-resident (loaded once).
Per iteration: AllReduce of partial attention scores [80,80], AllGather of
attention output slices, AllReduce of partial FF outputs. Only the first two
components of the 1026-wide fc head are computed (the rest is discarded).
"""
import sys
import types
import numpy as np

import concourse.bass as bass
import concourse.bacc as bacc
import concourse.tile as tile
import concourse.mybir as mybir

P = 128
N = 80           # boundary points (tokens per batch)
D = 1222         # token dim
DP = 1280        # padded token dim (10*128); col 1222 = constant-1 bias col
FF = 2048
H0 = W0 = 224
CH = 256         # channels per core
NCELL = 66640    # cells over all scales (50176+12544+3136+784)
NITER = 6
SCALE_HW = [(224, 224), (112, 112), (56, 56), (28, 28)]
SCALE_BASE = [0, 50176, 62720, 65856]

QS = 320         # per-core col slice of Q / K / V'
QSP = 384        # padded to 3*128 for transposes
FS = 512         # per-core col slice of lin1 / row slice of lin2
FSP = 640        # padded: col 512 = bias col (core 0 carries lin2 bias row)

F32 = mybir.dt.float32
BF16 = mybir.dt.bfloat16
I32 = mybir.dt.int32
AX = mybir.AxisListType
OP = mybir.AluOpType
AF = mybir.ActivationFunctionType

GROUPS = [[0, 1, 2, 3], [4, 5, 6, 7]]


def install_profile_hook():
    """Enable run_bass_kernel_spmd(trace=True) NTFF profiling (optional)."""
    try:
        import antenv
        if "antenv.axon_hooks" in sys.modules:
            return
        mod = types.ModuleType("antenv.axon_hooks")
        mod._hook = None
        mod.set_axon_ntff_profile_hook = lambda h: setattr(mod, "_hook", h)
        mod.get_axon_ntff_profile_hook = lambda: mod._hook
        sys.modules["antenv.axon_hooks"] = mod
        antenv.axon_hooks = mod
        from trn_agent_boot.trn_boot import _ntff_profile_via_ctypes
        mod._hook = _ntff_profile_via_ctypes("/opt/axon/libaxon_pjrt.so")
        import concourse.bass_utils as _bu
        _bu.upload_artifacts = lambda d: d
    except Exception:
        pass


# ---------------------------------------------------------------------------
# kernel build
# ---------------------------------------------------------------------------

def _bc(ap, shape):
    return ap.to_broadcast(shape)


def _ln(nc, sp, x_ap, n_feat, tag):
    """In-place LayerNorm over x_ap [N, n_feat] (gamma=1, beta=0, eps=1e-5)."""
    s = sp.tile([N, 1], F32, tag=tag + "m")
    nc.vector.tensor_reduce(out=s[:], in_=x_ap, op=OP.add, axis=AX.X)
    negm = sp.tile([N, 1], F32, tag=tag + "n")
    nc.vector.tensor_scalar(out=negm[:], in0=s[:], scalar1=-1.0 / n_feat,
                            scalar2=None, op0=OP.mult)
    sq = sp.tile([N, n_feat], F32, tag="lnsq")
    ssq = sp.tile([N, 1], F32, tag=tag + "s")
    nc.scalar.activation(out=sq[:], in_=x_ap, func=AF.Square,
                         bias=negm[:], accum_out=ssq[:])
    var = sp.tile([N, 1], F32, tag=tag + "v")
    nc.vector.tensor_scalar(out=var[:], in0=ssq[:], scalar1=1.0 / n_feat,
                            scalar2=1e-5, op0=OP.mult, op1=OP.add)
    sig = sp.tile([N, 1], F32, tag=tag + "g")
    nc.scalar.activation(out=sig[:], in_=var[:], func=AF.Sqrt)
    rstd = sp.tile([N, 1], F32, tag=tag + "r")
    nc.vector.reciprocal(out=rstd[:], in_=sig[:])
    nc.vector.tensor_scalar(out=x_ap, in0=x_ap, scalar1=negm[:],
                            scalar2=rstd[:], op0=OP.add, op1=OP.mult)


def _tp_cols(nc, sp, pq, src_ap, n_cols, ident, tag):
    """[N, n_cols] -> [128, n_cols/128, N] (feature blocks on partitions)."""
    nblk = n_cols // P
    xt = sp.tile([P, nblk, N], F32, tag=tag)
    for k in range(nblk):
        ps = pq.tile([P, N], F32, tag="tpps", space="PSUM")
        nc.tensor.transpose(out=ps[:], in_=src_ap[:, P * k:P * (k + 1)],
                            identity=ident[:N, :N])
        nc.vector.tensor_copy(out=xt[:, k, :], in_=ps[:])
    return xt


def build_kernel():
    nc = bacc.Bacc(None, target_bir_lowering=False)

    img = nc.dram_tensor("img", [CH, H0, W0], F32, kind="ExternalInput")
    bnd_in = nc.dram_tensor("bnd_in", [N, 2], I32, kind="ExternalInput")
    tbl_in = nc.dram_tensor("tbl_in", [N, 168], I32, kind="ExternalInput")
    msk_in = nc.dram_tensor("msk_in", [N, 400], I32, kind="ExternalInput")
    cst_in = nc.dram_tensor("cst_in", [N, 3 * D + 2], F32,
                            kind="ExternalInput")
    ident_in = nc.dram_tensor("ident_in", [P, P], F32, kind="ExternalInput")
    qkvw = nc.dram_tensor("qkvw", [DP, 3 * QS], F32, kind="ExternalInput")
    lin1w = nc.dram_tensor("lin1w", [DP, FS], F32, kind="ExternalInput")
    lin2w = nc.dram_tensor("lin2w", [FSP, D], F32, kind="ExternalInput")

    traj = nc.dram_tensor("traj", [NITER, N, 2], I32, kind="ExternalOutput")
    dbg_tok = nc.dram_tensor("dbg_tok", [N, D], F32, kind="ExternalOutput")
    dbg_x3 = nc.dram_tensor("dbg_x3", [N, D], F32, kind="ExternalOutput")
    dbg_off = nc.dram_tensor("dbg_off", [N, 2], F32, kind="ExternalOutput")

    with tile.TileContext(nc) as tc:
        with tc.tile_pool(name="dram", bufs=1, space="DRAM") as drp, \
             tc.tile_pool(name="cst", bufs=1) as cp, \
             tc.tile_pool(name="pp", bufs=2, space="PSUM") as pp, \
             tc.tile_pool(name="pq", bufs=2, space="PSUM") as pq, \
             tc.tile_pool(name="cc", bufs=2, space="DRAM") as ccp:

            maps = drp.tile([NCELL, CH], BF16)

            # resident weights (one big DMA each; overlap with preprocess)
            w_qkv = cp.tile([P, 10, 3 * QS], F32)
            nc.sync.dma_start(
                w_qkv[:], qkvw[:].rearrange("(k p) c -> p k c", p=P))
            w_l1 = cp.tile([P, 10, FS], F32)
            nc.sync.dma_start(
                w_l1[:], lin1w[:].rearrange("(k p) c -> p k c", p=P))
            w_l2 = cp.tile([P, 5, D], F32)
            nc.sync.dma_start(
                w_l2[:], lin2w[:].rearrange("(k p) c -> p k c", p=P))

            ident = cp.tile([P, P], F32)
            nc.sync.dma_start(ident[:], ident_in[:])
            tbl = cp.tile([N, 168], I32)
            nc.sync.dma_start(tbl[:], tbl_in[:])
            msk = cp.tile([N, 400], I32)
            nc.sync.dma_start(msk[:], msk_in[:])
            cst = cp.tile([N, 3 * D + 2], F32)
            nc.sync.dma_start(cst[:], cst_in[:])

            # ---------------- preprocess: build HWC pooled maps ----------
            _preprocess(nc, tc, maps, img, ident)

            # ---------------- iterations (pools reuse preprocess SBUF) ----
            with tc.tile_pool(name="it", bufs=1) as sp, \
                 tc.tile_pool(name="gat", bufs=3) as gp:
                _iterations(nc, tc, sp, gp, pp, pq, ccp, maps, bnd_in, tbl,
                            msk, cst, ident, w_qkv, w_l1, w_l2,
                            traj, dbg_tok, dbg_x3, dbg_off)
    nc.finalize()
    return nc


def _preprocess(nc, tc, maps, img, ident):
    """CHW -> cell-major HWC bf16 maps via XBAR DMA transpose (no PE)."""
    maps_t = maps  # [NCELL, CH] DRAM tile

    def store_blocks(T_ap, nblk, base, cc):
        dst = bass.AP(tensor=maps_t.tensor, offset=base * CH + cc * P,
                      ap=[[CH, P], [P * CH, nblk], [1, P]])
        nc.sync.dma_start(dst, T_ap)

    def store_rem(T_ap, rem, base, cc):
        dst = bass.AP(tensor=maps_t.tensor, offset=base * CH + cc * P,
                      ap=[[CH, rem], [1, P]])
        nc.sync.dma_start(dst, T_ap)

    with tc.tile_pool(name="ppin", bufs=3) as pin, \
         tc.tile_pool(name="ppst", bufs=3) as pst:
        for hg in range(28):
            for cc in range(2):
                A = pin.tile([P, 8 * W0], F32, tag="A")
                nc.sync.dma_start(
                    A[:], img[P * cc:P * (cc + 1), 8 * hg:8 * hg + 8, :])
                A3 = A[:].rearrange("p (h w) -> p h w", w=W0)
                A1 = pin.tile([P, 448], F32, tag="A1")
                A13 = A1[:].rearrange("p (h w) -> p h w", w=112)
                nc.vector.tensor_tensor(out=A13, in0=A3[:, 0::2, 0::2],
                                        in1=A3[:, 0::2, 1::2], op=OP.add)
                nc.vector.tensor_tensor(out=A13, in0=A13,
                                        in1=A3[:, 1::2, 0::2], op=OP.add)
                nc.vector.tensor_tensor(out=A13, in0=A13,
                                        in1=A3[:, 1::2, 1::2], op=OP.add)
                nc.vector.tensor_scalar(out=A13, in0=A13, scalar1=0.25,
                                        scalar2=None, op0=OP.mult)
                A2 = pin.tile([P, 112], F32, tag="A2")
                A23 = A2[:].rearrange("p (h w) -> p h w", w=56)
                nc.vector.tensor_tensor(out=A23, in0=A13[:, 0::2, 0::2],
                                        in1=A13[:, 0::2, 1::2], op=OP.add)
                nc.vector.tensor_tensor(out=A23, in0=A23,
                                        in1=A13[:, 1::2, 0::2], op=OP.add)
                nc.vector.tensor_tensor(out=A23, in0=A23,
                                        in1=A13[:, 1::2, 1::2], op=OP.add)
                nc.vector.tensor_scalar(out=A23, in0=A23, scalar1=0.25,
                                        scalar2=None, op0=OP.mult)
                A4 = pin.tile([P, 28], F32, tag="A4")
                A43 = A4[:].rearrange("p (h w) -> p h w", w=28)
                nc.vector.tensor_tensor(out=A43, in0=A23[:, 0::2, 0::2],
                                        in1=A23[:, 0::2, 1::2], op=OP.add)
                nc.vector.tensor_tensor(out=A43, in0=A43,
                                        in1=A23[:, 1::2, 0::2], op=OP.add)
                nc.vector.tensor_tensor(out=A43, in0=A43,
                                        in1=A23[:, 1::2, 1::2], op=OP.add)
                nc.vector.tensor_scalar(out=A43, in0=A43, scalar1=0.25,
                                        scalar2=None, op0=OP.mult)

                # cast to bf16 (scalar engine; DVE stays on pooling)
                Ab = pin.tile([P, 1792], BF16, tag="Ab")
                nc.scalar.activation(out=Ab[:], in_=A[:], func=AF.Copy)
                A1b = pin.tile([P, 512], BF16, tag="A1b")
                nc.scalar.activation(out=A1b[:, 0:448], in_=A1[:],
                                     func=AF.Copy)
                A2b = pin.tile([P, 128], BF16, tag="A2b")
                nc.scalar.activation(out=A2b[:, 0:112], in_=A2[:],
                                     func=AF.Copy)
                A4b = pin.tile([P, 128], BF16, tag="A4b")
                nc.scalar.activation(out=A4b[:, 0:28], in_=A4[:],
                                     func=AF.Copy)

                # XBAR transposes: out[p, k, c] = in[c, k*128 + p]
                T0 = pst.tile([P, 14, P], BF16, tag="T0")
                nc.sync.dma_start_transpose(T0[:], Ab[:])
                T1 = pst.tile([P, 4, P], BF16, tag="T1")
                nc.sync.dma_start_transpose(T1[:], A1b[:])
                T2 = pst.tile([P, P], BF16, tag="T2")
                nc.sync.dma_start_transpose(T2[:], A2b[:])
                T4 = pst.tile([P, P], BF16, tag="T4")
                nc.sync.dma_start_transpose(T4[:], A4b[:])

                store_blocks(T0[:], 14, hg * 1792, cc)
                store_blocks(T1[:, 0:3, :], 3, 50176 + hg * 448, cc)
                store_rem(T1[0:64, 3, :], 64, 50176 + hg * 448 + 384, cc)
                store_rem(T2[0:112, :], 112, 62720 + hg * 112, cc)
                store_rem(T4[0:28, :], 28, 65856 + hg * 28, cc)


def _iterations(nc, tc, sp, gp, pp, pq, ccp, maps, bnd_in, tbl, msk, cst,
                ident, w_qkv, w_l1, w_l2, traj, dbg_tok, dbg_x3, dbg_off):
    maps_flat = maps[:].rearrange("c e -> (c e)").rearrange(
        "(a b) -> a b", b=1)
    pe_ap = cst[:, 0:D]
    fcw0 = cst[:, D:2 * D]
    fcw1 = cst[:, 2 * D:3 * D]
    fcb0 = cst[:, 3 * D:3 * D + 1]
    fcb1 = cst[:, 3 * D + 1:3 * D + 2]
    inv_sqrt_d = 1.0 / float(np.sqrt(D))

    # persistent tiles (padded regions initialized once)
    bnd = sp.tile([N, 2], I32, tag="bnd")
    nc.sync.dma_start(bnd[:], bnd_in[:])
    tok = sp.tile([N, DP], F32, tag="tok")
    x2 = sp.tile([N, DP], F32, tag="x2")
    q_t = sp.tile([N, QSP], F32, tag="q_t")
    k_t = sp.tile([N, QSP], F32, tag="k_t")
    v_t = sp.tile([N, QS], F32, tag="v_t")
    h = sp.tile([N, FSP], F32, tag="h")
    for t, c in ((tok, D), (h, FS)):
        nc.vector.memset(t[:], 0.0)
        nc.vector.memset(t[:, c:c + 1], 1.0)
    nc.vector.memset(q_t[:], 0.0)   # cols 320..384 stay zero
    nc.vector.memset(k_t[:], 0.0)

    for it in range(NITER):
        # ---- gather indices [N, 4, 7] ----
        bsh = sp.tile([N, 8], I32, tag="bsh")
        nc.vector.tensor_tensor(
            out=bsh[:].rearrange("n (a s) -> n a s", a=2),
            in0=_bc(bnd[:].rearrange("n (a s) -> n a s", s=1), [N, 2, 4]),
            in1=_bc(tbl[:, 140:144].rearrange("n (a s) -> n a s", a=1),
                    [N, 2, 4]),
            op=OP.arith_shift_right)
        bx7 = _bc(bsh[:, 0:4].rearrange("n (s a) -> n s a", a=1), [N, 4, 7])
        by7 = _bc(bsh[:, 4:8].rearrange("n (s a) -> n s a", a=1), [N, 4, 7])
        idx = sp.tile([N, 28], I32, tag="idx")
        idx3 = idx[:].rearrange("n (s d) -> n s d", s=4)
        tbl3 = tbl[:].rearrange("n (g c) -> n g c", c=28)
        nc.vector.tensor_tensor(
            out=idx3, in0=bx7,
            in1=tbl3[:, 0, :].rearrange("n (s d) -> n s d", s=4), op=OP.add)
        nc.vector.tensor_scalar(out=idx[:], in0=idx[:], scalar1=0,
                                scalar2=None, op0=OP.max)
        nc.vector.tensor_tensor(
            out=idx3, in0=idx3,
            in1=tbl3[:, 1, :].rearrange("n (s d) -> n s d", s=4), op=OP.min)
        nc.vector.tensor_tensor(
            out=idx3, in0=idx3,
            in1=tbl3[:, 2, :].rearrange("n (s d) -> n s d", s=4), op=OP.mult)
        nc.vector.tensor_tensor(out=idx3, in0=idx3, in1=by7, op=OP.add)
        nc.vector.tensor_scalar(out=idx[:], in0=idx[:], scalar1=-3,
                                scalar2=0, op0=OP.add, op1=OP.max)
        nc.vector.tensor_tensor(
            out=idx3, in0=idx3,
            in1=tbl3[:, 4, :].rearrange("n (s d) -> n s d", s=4), op=OP.min)
        nc.vector.tensor_tensor(
            out=idx3, in0=idx3,
            in1=tbl3[:, 3, :].rearrange("n (s d) -> n s d", s=4), op=OP.add)
        nc.vector.tensor_scalar(out=idx[:], in0=idx[:], scalar1=CH,
                                scalar2=None, op0=OP.mult)

        # ---- masks [N, 196] ----
        bx49 = _bc(bsh[:, 0:4].rearrange("n (s a) -> n s a", a=1), [N, 4, 49])
        by49 = _bc(bsh[:, 4:8].rearrange("n (s a) -> n s a", a=1), [N, 4, 49])
        m3 = lambda t: t.rearrange("n (s d) -> n s d", s=4)
        mi = sp.tile([N, 196], I32, tag="mi")
        mt = sp.tile([N, 196], I32, tag="mt")
        hs49 = _bc(msk[:, 392:396].rearrange("n (s a) -> n s a", a=1),
                   [N, 4, 49])
        ws49 = _bc(msk[:, 396:400].rearrange("n (s a) -> n s a", a=1),
                   [N, 4, 49])
        nc.vector.tensor_tensor(out=m3(mt[:]), in0=bx49,
                                in1=m3(msk[:, 0:196]), op=OP.add)
        nc.vector.tensor_tensor(out=m3(mi[:]), in0=m3(mt[:]), in1=hs49,
                                op=OP.is_le)
        nc.vector.tensor_scalar(out=mt[:], in0=mt[:], scalar1=0,
                                scalar2=None, op0=OP.is_ge)
        nc.vector.tensor_tensor(out=mi[:], in0=mi[:], in1=mt[:],
                                op=OP.bitwise_and)
        nc.vector.tensor_tensor(out=m3(mt[:]), in0=by49,
                                in1=m3(msk[:, 196:392]), op=OP.add)
        mw = sp.tile([N, 196], I32, tag="mw")
        nc.vector.tensor_tensor(out=m3(mw[:]), in0=m3(mt[:]), in1=ws49,
                                op=OP.is_le)
        nc.vector.tensor_tensor(out=mi[:], in0=mi[:], in1=mw[:],
                                op=OP.bitwise_and)
        nc.vector.tensor_scalar(out=mt[:], in0=mt[:], scalar1=0,
                                scalar2=None, op0=OP.is_ge)
        nc.vector.tensor_tensor(out=mi[:], in0=mi[:], in1=mt[:],
                                op=OP.bitwise_and)
        mask = sp.tile([N, 196], F32, tag="mask")
        nc.vector.tensor_copy(out=mask[:], in_=mi[:])

        # ---- gathers + dots (per 7-cell run; qf run gathered first) ----
        dots = sp.tile([N, 196], F32, tag="dots")
        qf = sp.tile([N, CH], F32, tag="qf")
        run_order = [(0, 3), (0, 0), (0, 1), (0, 2), (0, 4), (0, 5), (0, 6)]
        run_order += [(s, dx) for s in range(1, 4) for dx in range(7)]
        qfb = sp.tile([N, CH], BF16, tag="qfb")
        for (s, dx) in run_order:
            K = gp.tile([N, 7 * CH], BF16, tag="K")
            nc.gpsimd.indirect_dma_start(
                out=K[:], out_offset=None, in_=maps_flat,
                in_offset=bass.IndirectOffsetOnAxis(
                    ap=idx[:, s * 7 + dx:s * 7 + dx + 1], axis=0))
            K3 = K[:].rearrange("n (d c) -> n d c", c=CH)
            if s == 0 and dx == 3:
                nc.vector.tensor_copy(out=qf[:], in_=K3[:, 3, :])
                nc.vector.tensor_copy(out=qfb[:], in_=K3[:, 3, :])
            Kf = sp.tile([N, 7 * CH], F32, tag="Kf")
            Kf3 = Kf[:].rearrange("n (d c) -> n d c", c=CH)
            nc.vector.tensor_tensor(
                out=Kf3, in0=K3,
                in1=_bc(qfb[:].rearrange("n (a c) -> n a c", a=1),
                        [N, 7, CH]),
                op=OP.mult)
            j = s * 49 + dx * 7
            nc.vector.tensor_reduce(
                out=dots[:, j:j + 7].rearrange("n (d a) -> n d a", a=1),
                in_=Kf3, op=OP.add, axis=AX.X)
        nc.vector.tensor_tensor(out=dots[:], in0=dots[:], in1=mask[:],
                                op=OP.mult)

        # ---- AllGather (qf | dots) ----
        agi = sp.tile([N, CH + 196], F32, tag="agi")
        nc.vector.tensor_copy(out=agi[:, 0:CH], in_=qf[:])
        nc.vector.tensor_copy(out=agi[:, CH:], in_=dots[:])
        cin = ccp.tile([N, CH + 196], F32, tag="cin")
        cout = ccp.tile([4 * N, CH + 196], F32, tag="cout")
        nc.sync.dma_start(cin[:], agi[:])
        nc.gpsimd.collective_compute(
            "AllGather", OP.bypass, ins=[cin[:]], outs=[cout[:]],
            replica_groups=GROUPS)

        # ---- tokens (single rearranged DMA for the qf columns) ----
        nc.sync.dma_start(
            tok[:, 0:1024].rearrange("n (r c) -> n r c", c=CH),
            cout[:].rearrange("(r n) e -> r n e", n=N)
            .rearrange("r n e -> n r e")[:, :, 0:CH])
        dsum = sp.tile([N, 4, 196], F32, tag="dsum")
        nc.sync.dma_start(
            dsum[:], cout[:].rearrange("(r n) e -> r n e", n=N)
            .rearrange("r n e -> n r e")[:, :, CH:])
        nc.vector.tensor_tensor(out=dsum[:, 0, :], in0=dsum[:, 0, :],
                                in1=dsum[:, 1, :], op=OP.add)
        nc.vector.tensor_tensor(out=dsum[:, 2, :], in0=dsum[:, 2, :],
                                in1=dsum[:, 3, :], op=OP.add)
        nc.vector.tensor_tensor(out=tok[:, 1024:1220], in0=dsum[:, 0, :],
                                in1=dsum[:, 2, :], op=OP.add)
        nc.vector.tensor_copy(out=tok[:, 1220:1222], in_=bnd[:])
        _ln(nc, sp, tok[:, 0:D], D, "l1")
        nc.vector.tensor_tensor(out=tok[:, 0:D], in0=tok[:, 0:D],
                                in1=pe_ap, op=OP.add)
        if it == 0:
            nc.sync.dma_start(dbg_tok[:], tok[:, 0:D])

        # ---- QKV slices (out_proj folded into V), resident weights ----
        xt = _tp_cols(nc, sp, pq, tok[:], DP, ident, "xt")
        for ci, dst in ((0, q_t), (1, k_t), (2, v_t)):
            ps = pp.tile([N, QS], F32, tag="mmps", space="PSUM")
            for k in range(10):
                nc.tensor.matmul(ps[:], xt[:, k, :],
                                 w_qkv[:, k, QS * ci:QS * (ci + 1)],
                                 start=(k == 0), stop=(k == 9))
            nc.vector.tensor_copy(out=dst[:, 0:QS], in_=ps[:])

        # ---- attention scores: partial over this core's 320 cols ----
        qT = _tp_cols(nc, sp, pq, q_t[:], QSP, ident, "qT")
        kT = _tp_cols(nc, sp, pq, k_t[:], QSP, ident, "kT")
        sc_ps = pp.tile([N, N], F32, tag="mmps", space="PSUM")
        for k in range(3):
            nc.tensor.matmul(sc_ps[:], qT[:, k, :], kT[:, k, :],
                             start=(k == 0), stop=(k == 2))
        sc = sp.tile([N, N], F32, tag="sc")
        nc.vector.tensor_scalar(out=sc[:], in0=sc_ps[:], scalar1=inv_sqrt_d,
                                scalar2=None, op0=OP.mult)
        sin_ = ccp.tile([N, N], F32, tag="sin")
        sout = ccp.tile([4 * N, N], F32, tag="sout")
        nc.sync.dma_start(sin_[:], sc[:])
        nc.gpsimd.collective_compute(
            "AllGather", OP.bypass, ins=[sin_[:]], outs=[sout[:]],
            replica_groups=GROUPS)
        scg = sp.tile([N, 4, N], F32, tag="scg")
        nc.sync.dma_start(
            scg[:], sout[:].rearrange("(r n) e -> r n e", n=N)
            .rearrange("r n e -> n r e"))
        nc.vector.tensor_tensor(out=scg[:, 0, :], in0=scg[:, 0, :],
                                in1=scg[:, 1, :], op=OP.add)
        nc.vector.tensor_tensor(out=scg[:, 2, :], in0=scg[:, 2, :],
                                in1=scg[:, 3, :], op=OP.add)
        nc.vector.tensor_tensor(out=sc[:], in0=scg[:, 0, :],
                                in1=scg[:, 2, :], op=OP.add)

        # ---- softmax (replicated) ----
        mx = sp.tile([N, 1], F32, tag="mx")
        nc.vector.tensor_reduce(out=mx[:], in_=sc[:], op=OP.max, axis=AX.X)
        nmx = sp.tile([N, 1], F32, tag="nmx")
        nc.vector.tensor_scalar(out=nmx[:], in0=mx[:], scalar1=-1.0,
                                scalar2=None, op0=OP.mult)
        esum = sp.tile([N, 1], F32, tag="esum")
        nc.scalar.activation(out=sc[:], in_=sc[:], func=AF.Exp,
                             bias=nmx[:], accum_out=esum[:])
        rsum = sp.tile([N, 1], F32, tag="rsum")
        nc.vector.reciprocal(out=rsum[:], in_=esum[:])
        nc.vector.tensor_scalar(out=sc[:], in0=sc[:], scalar1=rsum[:],
                                scalar2=None, op0=OP.mult)
        smT_ps = pq.tile([N, N], F32, tag="tpps", space="PSUM")
        nc.tensor.transpose(out=smT_ps[:], in_=sc[:], identity=ident[:N, :N])
        smT = sp.tile([N, N], F32, tag="smT")
        nc.vector.tensor_copy(out=smT[:], in_=smT_ps[:])

        # ---- attention output slice [N, 320] + AllGather ----
        at_ps = pp.tile([N, QS], F32, tag="mmps", space="PSUM")
        nc.tensor.matmul(at_ps[:], smT[:], v_t[:], start=True, stop=True)
        at_s = sp.tile([N, QS], F32, tag="at_s")
        nc.vector.tensor_copy(out=at_s[:], in_=at_ps[:])
        ain = ccp.tile([N, QS], F32, tag="ain")
        aout = ccp.tile([4 * N, QS], F32, tag="aout")
        nc.sync.dma_start(ain[:], at_s[:])
        nc.gpsimd.collective_compute(
            "AllGather", OP.bypass, ins=[ain[:]], outs=[aout[:]],
            replica_groups=GROUPS)
        nc.sync.dma_start(
            x2[:].rearrange("n (r c) -> n r c", c=QS),
            aout[:].rearrange("(r n) c -> r n c", n=N)
            .rearrange("r n c -> n r c"))
        # x2 = attn + tok over full padded width (restores bias col 1222)
        nc.vector.tensor_tensor(out=x2[:], in0=x2[:], in1=tok[:], op=OP.add)
        _ln(nc, sp, x2[:, 0:D], D, "l2")

        # ---- FF: lin1 col-slice -> relu -> lin2 row-slice (partial) ----
        x2T = _tp_cols(nc, sp, pq, x2[:], DP, ident, "x2T")
        ps1 = pp.tile([N, FS], F32, tag="mmps", space="PSUM")
        for k in range(10):
            nc.tensor.matmul(ps1[:], x2T[:, k, :], w_l1[:, k, :],
                             start=(k == 0), stop=(k == 9))
        nc.vector.tensor_scalar(out=h[:, 0:FS], in0=ps1[:],
                                scalar1=0.0, scalar2=None, op0=OP.max)
        hT = _tp_cols(nc, sp, pq, h[:], FSP, ident, "hT")
        x3 = sp.tile([N, D], F32, tag="x3")
        for ccol in range(3):
            c0 = 512 * ccol
            cw = min(512, D - c0)
            ps = pp.tile([N, 512], F32, tag="mmps", space="PSUM")
            for k in range(5):
                nc.tensor.matmul(ps[:, :cw], hT[:, k, :],
                                 w_l2[:, k, c0:c0 + cw],
                                 start=(k == 0), stop=(k == 4))
            nc.vector.tensor_copy(out=x3[:, c0:c0 + cw], in_=ps[:, :cw])
        xin = ccp.tile([N, D], F32, tag="xin")
        xout = ccp.tile([4 * N, D], F32, tag="xout")
        nc.sync.dma_start(xin[:], x3[:])
        nc.gpsimd.collective_compute(
            "AllGather", OP.bypass, ins=[xin[:]], outs=[xout[:]],
            replica_groups=GROUPS)
        xg = sp.tile([N, 2, D], F32, tag="xg")
        xo4 = xout[:].rearrange("(r n) e -> r n e", n=N) \
            .rearrange("r n e -> n r e")
        nc.sync.dma_start(xg[:], xo4[:, 0:2, :])
        nc.vector.tensor_tensor(out=x3[:], in0=xg[:, 0, :],
                                in1=xg[:, 1, :], op=OP.add)
        nc.sync.dma_start(xg[:], xo4[:, 2:4, :])
        nc.vector.tensor_tensor(out=xg[:, 0, :], in0=xg[:, 0, :],
                                in1=xg[:, 1, :], op=OP.add)
        nc.vector.tensor_tensor(out=x3[:], in0=x3[:], in1=xg[:, 0, :],
                                op=OP.add)
        nc.vector.tensor_tensor(out=x3[:], in0=x3[:], in1=x2[:, 0:D],
                                op=OP.add)
        _ln(nc, sp, x3[:], D, "l3")
        if it == 0:
            nc.sync.dma_start(dbg_x3[:], x3[:])

        # ---- fc head (only 2 outputs) ----
        f0 = sp.tile([N, D], F32, tag="lnsq")
        off = sp.tile([N, 2], F32, tag="off")
        nc.vector.tensor_tensor(out=f0[:], in0=x3[:], in1=fcw0, op=OP.mult)
        nc.vector.tensor_reduce(out=off[:, 0:1], in_=f0[:], op=OP.add,
                                axis=AX.X)
        nc.vector.tensor_tensor(out=f0[:], in0=x3[:], in1=fcw1, op=OP.mult)
        nc.vector.tensor_reduce(out=off[:, 1:2], in_=f0[:], op=OP.add,
                                axis=AX.X)
        nc.vector.tensor_tensor(out=off[:], in0=off[:],
                                in1=cst[:, 3 * D:3 * D + 2], op=OP.add)
        if it == 0:
            nc.sync.dma_start(dbg_off[:], off[:])

        # trunc toward zero: rne(off - 0.5*sign(off)); exact ints unaffected
        sgn = sp.tile([N, 2], F32, tag="sgn")
        nc.scalar.activation(out=sgn[:], in_=off[:], func=AF.Sign)
        nc.vector.tensor_scalar(out=sgn[:], in0=sgn[:], scalar1=-0.5,
                                scalar2=None, op0=OP.mult)
        nc.vector.tensor_tensor(out=off[:], in0=off[:], in1=sgn[:],
                                op=OP.add)
        ti = sp.tile([N, 2], I32, tag="ti")
        nc.vector.tensor_copy(out=ti[:], in_=off[:])
        nc.vector.tensor_tensor(out=bnd[:], in0=bnd[:], in1=ti[:], op=OP.add)
        nc.vector.tensor_scalar(out=bnd[:], in0=bnd[:], scalar1=0,
                                scalar2=223, op0=OP.max, op1=OP.min)
        nc.sync.dma_start(traj[it, :, :], bnd[:])


# ---------------------------------------------------------------------------
# host side
# ---------------------------------------------------------------------------

_NC_CACHE = {}


def _host_inputs(curr_img_features, previous_boundary, in_proj_w, in_proj_b,
                 out_proj_w, out_proj_b, lin1_w, lin1_b, lin2_w, lin2_b,
                 fc_w, fc_b):
    f32 = np.float32
    pos = np.arange(N, dtype=f32)[:, None]
    div = np.exp(np.arange(0, D, 2, dtype=f32) * (-np.log(10000.0) / D))
    pe = np.zeros((N, D), f32)
    pe[:, 0::2] = np.sin(pos * div)
    pe[:, 1::2] = np.cos(pos * div)

    Wq, Wk, Wv = (np.asarray(in_proj_w[i * D:(i + 1) * D], f32)
                  for i in range(3))
    bq, bk, bv = (np.asarray(in_proj_b[i * D:(i + 1) * D], f32)
                  for i in range(3))
    Wvp = np.asarray(out_proj_w, f32) @ Wv          # [D, D]
    bvp = np.asarray(out_proj_w, f32) @ bv + np.asarray(out_proj_b, f32)

    # padded [DP, 3*DP_cols]: rows = input dim (row D = bias), cols sharded
    qkv_full = np.zeros((DP, 3 * DP), f32)
    for i, (W, b) in enumerate(((Wq, bq), (Wk, bk), (Wvp, bvp))):
        qkv_full[0:D, DP * i:DP * i + D] = W.T
        qkv_full[D, DP * i:DP * i + D] = b

    l1 = np.zeros((DP, FF), f32)
    l1[0:D, :] = np.asarray(lin1_w, f32).T
    l1[D, :] = np.asarray(lin1_b, f32)
    l2t = np.asarray(lin2_w, f32).T                  # [FF, D]
    l2b = np.asarray(lin2_b, f32)

    cst = np.zeros((N, 3 * D + 2), f32)
    cst[:, 0:D] = pe
    cst[:, D:2 * D] = np.asarray(fc_w[:, 0, :], f32)
    cst[:, 2 * D:3 * D] = np.asarray(fc_w[:, 1, :], f32)
    cst[:, 3 * D:3 * D + 2] = np.asarray(fc_b[:, :2], f32)

    tbl = np.zeros((168,), np.int32)
    for s in range(4):
        Hs, Ws = SCALE_HW[s]
        for dx in range(7):
            j = s * 7 + dx
            tbl[j] = dx - 3
            tbl[28 + j] = Hs - 1
            tbl[56 + j] = Ws
            tbl[84 + j] = SCALE_BASE[s]
            tbl[112 + j] = Hs * Ws - 7
    tbl[140:144] = [0, 1, 2, 3]
    tblr = np.tile(tbl[None, :], (N, 1))

    mskv = np.zeros((400,), np.int32)
    for s in range(4):
        Hs, Ws = SCALE_HW[s]
        for dx in range(7):
            for dy in range(7):
                j = s * 49 + dx * 7 + dy
                mskv[j] = dx - 3
                mskv[196 + j] = dy - 3
        mskv[392 + s] = Hs - 1
        mskv[396 + s] = Ws - 1
    mskr = np.tile(mskv[None, :], (N, 1))

    ident = np.eye(P, dtype=f32)

    imgs = np.asarray(curr_img_features, f32)
    bnds = np.asarray(previous_boundary, np.int32)
    in_maps = []
    for c in range(8):
        g, q = c // 4, c % 4
        # Q/K/V col slice for this core
        qs = np.ascontiguousarray(np.concatenate(
            [qkv_full[:, DP * i + QS * q:DP * i + QS * (q + 1)]
             for i in range(3)], axis=1))           # [DP, 960]
        l1s = np.ascontiguousarray(l1[:, FS * q:FS * (q + 1)])  # [DP, 512]
        l2s = np.zeros((FSP, D), f32)
        l2s[0:FS, :] = l2t[FS * q:FS * (q + 1), :]
        if q == 0:
            l2s[FS, :] = l2b                         # bias row (once)
        m = dict(tbl_in=tblr, msk_in=mskr, cst_in=cst, ident_in=ident,
                 qkvw=qs, lin1w=l1s, lin2w=l2s)
        m["img"] = np.ascontiguousarray(imgs[g, CH * q:CH * (q + 1)])
        m["bnd_in"] = np.ascontiguousarray(bnds[g])
        in_maps.append(m)
    return in_maps


def make_in_maps(inputs):
    return _host_inputs(
        inputs["curr_img_features"], inputs["previous_boundary"],
        inputs["in_proj_w"], inputs["in_proj_b"],
        inputs["out_proj_w"], inputs["out_proj_b"],
        inputs["lin1_w"], inputs["lin1_b"],
        inputs["lin2_w"], inputs["lin2_b"],
        inputs["fc_w"], inputs["fc_b"])


def kernel(**inputs):
    from concourse.bass_utils import run_bass_kernel_spmd
    install_profile_hook()

    in_maps = make_in_maps(inputs)
    if "nc" not in _NC_CACHE:
        _NC_CACHE["nc"] = build_kernel()
    nc = _NC_CACHE["nc"]
    res = run_bass_kernel_spmd(nc, in_maps, core_ids=list(range(8)))
    kernel.last_results = res
    t0 = res.results[0]["traj"]   # batch 0
    t1 = res.results[4]["traj"]   # batch 1
    return np.stack([t0, t1], axis=1).astype(np.int32)  # [6, 2, 80, 2]


# revision 21
# speedup vs baseline: 1.0941x; 1.0941x over previous
"""Trainium2 Bass kernel for nn_NeighborModel (boundary refinement w/ sparse
neighborhood attention), SPMD over 8 NeuronCores.

Sharding: 2 groups x 4 cores; group g owns batch g; core q of a group owns a
256-channel chunk of that batch's feature maps (4 avg-pooled scales, cell-major
HWC layout, built on device). Each core gathers 7x7 neighborhoods around all
80 boundary points (7-cell runs via indirect DMA with flat element indices),
computes partial dot-products on DVE, then one AllGather per iteration
exchanges (qf chunk | partial dots). The transformer layer (80 tokens) is
tensor-parallel across the 4 cores of a group: core q owns a 320-column slice
of Q/K/V' (out_proj folded into V), a 512-column slice of lin1, and the
matching 512-row slice of lin2. All weights are SBUF-resident (loaded once).
Per iteration: AllReduce of partial attention scores [80,80], AllGather of
attention output slices, AllReduce of partial FF outputs. Only the first two
components of the 1026-wide fc head are computed (the rest is discarded).
"""
import sys
import types
import numpy as np

import concourse.bass as bass
import concourse.bacc as bacc
import concourse.tile as tile
import concourse.mybir as mybir

P = 128
N = 80           # boundary points (tokens per batch)
D = 1222         # token dim
DP = 1280        # padded token dim (10*128); col 1222 = constant-1 bias col
FF = 2048
H0 = W0 = 224
CH = 256         # channels per core
NCELL = 66640    # cells over all scales (50176+12544+3136+784)
NITER = 6
SCALE_HW = [(224, 224), (112, 112), (56, 56), (28, 28)]
SCALE_BASE = [0, 50176, 62720, 65856]

QS = 320         # per-core col slice of Q / K / V'
QSP = 384        # padded to 3*128 for transposes
FS = 512         # per-core col slice of lin1 / row slice of lin2
FSP = 640        # padded: col 512 = bias col (core 0 carries lin2 bias row)

F32 = mybir.dt.float32
BF16 = mybir.dt.bfloat16
I32 = mybir.dt.int32
AX = mybir.AxisListType
OP = mybir.AluOpType
AF = mybir.ActivationFunctionType

GROUPS = [[0, 1, 2, 3], [4, 5, 6, 7]]


def install_profile_hook():
    """Enable run_bass_kernel_spmd(trace=True) NTFF profiling (optional)."""
    try:
        import antenv
        if "antenv.axon_hooks" in sys.modules:
            return
        mod = types.ModuleType("antenv.axon_hooks")
        mod._hook = None
        mod.set_axon_ntff_profile_hook = lambda h: setattr(mod, "_hook", h)
        mod.get_axon_ntff_profile_hook = lambda: mod._hook
        sys.modules["antenv.axon_hooks"] = mod
        antenv.axon_hooks = mod
        from trn_agent_boot.trn_boot import _ntff_profile_via_ctypes
        mod._hook = _ntff_profile_via_ctypes("/opt/axon/libaxon_pjrt.so")
        import concourse.bass_utils as _bu
        _bu.upload_artifacts = lambda d: d
    except Exception:
        pass


# ---------------------------------------------------------------------------
# kernel build
# ---------------------------------------------------------------------------

def _bc(ap, shape):
    return ap.to_broadcast(shape)


def _ln(nc, sp, x_ap, n_feat, tag):
    """In-place LayerNorm over x_ap [N, n_feat] (gamma=1, beta=0, eps=1e-5)."""
    s = sp.tile([N, 1], F32, tag=tag + "m")
    nc.vector.tensor_reduce(out=s[:], in_=x_ap, op=OP.add, axis=AX.X)
    negm = sp.tile([N, 1], F32, tag=tag + "n")
    nc.vector.tensor_scalar(out=negm[:], in0=s[:], scalar1=-1.0 / n_feat,
                            scalar2=None, op0=OP.mult)
    sq = sp.tile([N, n_feat], F32, tag="lnsq")
    ssq = sp.tile([N, 1], F32, tag=tag + "s")
    nc.scalar.activation(out=sq[:], in_=x_ap, func=AF.Square,
                         bias=negm[:], accum_out=ssq[:])
    var = sp.tile([N, 1], F32, tag=tag + "v")
    nc.vector.tensor_scalar(out=var[:], in0=ssq[:], scalar1=1.0 / n_feat,
                            scalar2=1e-5, op0=OP.mult, op1=OP.add)
    sig = sp.tile([N, 1], F32, tag=tag + "g")
    nc.scalar.activation(out=sig[:], in_=var[:], func=AF.Sqrt)
    rstd = sp.tile([N, 1], F32, tag=tag + "r")
    nc.vector.reciprocal(out=rstd[:], in_=sig[:])
    nc.vector.tensor_scalar(out=x_ap, in0=x_ap, scalar1=negm[:],
                            scalar2=rstd[:], op0=OP.add, op1=OP.mult)


def _tp_cols(nc, sp, pq, src_ap, n_cols, ident, tag):
    """[N, n_cols] -> [128, n_cols/128, N] (feature blocks on partitions)."""
    nblk = n_cols // P
    xt = sp.tile([P, nblk, N], F32, tag=tag)
    for k in range(nblk):
        ps = pq.tile([P, N], F32, tag="tpps", space="PSUM")
        nc.tensor.transpose(out=ps[:], in_=src_ap[:, P * k:P * (k + 1)],
                            identity=ident[:N, :N])
        nc.vector.tensor_copy(out=xt[:, k, :], in_=ps[:])
    return xt


def build_kernel():
    nc = bacc.Bacc(None, target_bir_lowering=False)

    img = nc.dram_tensor("img", [CH, H0, W0], F32, kind="ExternalInput")
    bnd_in = nc.dram_tensor("bnd_in", [N, 2], I32, kind="ExternalInput")
    tbl_in = nc.dram_tensor("tbl_in", [N, 168], I32, kind="ExternalInput")
    msk_in = nc.dram_tensor("msk_in", [N, 400], I32, kind="ExternalInput")
    cst_in = nc.dram_tensor("cst_in", [N, 3 * D + 2], F32,
                            kind="ExternalInput")
    ident_in = nc.dram_tensor("ident_in", [P, P], F32, kind="ExternalInput")
    qkvw = nc.dram_tensor("qkvw", [DP, 3 * QS], F32, kind="ExternalInput")
    lin1w = nc.dram_tensor("lin1w", [DP, FS], F32, kind="ExternalInput")
    lin2w = nc.dram_tensor("lin2w", [FSP, D], F32, kind="ExternalInput")

    traj = nc.dram_tensor("traj", [NITER, N, 2], I32, kind="ExternalOutput")
    dbg_tok = nc.dram_tensor("dbg_tok", [N, D], F32, kind="ExternalOutput")
    dbg_x3 = nc.dram_tensor("dbg_x3", [N, D], F32, kind="ExternalOutput")
    dbg_off = nc.dram_tensor("dbg_off", [N, 2], F32, kind="ExternalOutput")

    with tile.TileContext(nc) as tc:
        with tc.tile_pool(name="dram", bufs=1, space="DRAM") as drp, \
             tc.tile_pool(name="cst", bufs=1) as cp, \
             tc.tile_pool(name="pp", bufs=2, space="PSUM") as pp, \
             tc.tile_pool(name="pq", bufs=2, space="PSUM") as pq, \
             tc.tile_pool(name="cc", bufs=2, space="DRAM") as ccp:

            maps = drp.tile([NCELL, CH], BF16)

            # resident weights (one big DMA each; overlap with preprocess)
            w_qkv = cp.tile([P, 10, 3 * QS], F32)
            nc.sync.dma_start(
                w_qkv[:], qkvw[:].rearrange("(k p) c -> p k c", p=P))
            w_l1 = cp.tile([P, 10, FS], F32)
            nc.sync.dma_start(
                w_l1[:], lin1w[:].rearrange("(k p) c -> p k c", p=P))
            w_l2 = cp.tile([P, 5, D], F32)
            nc.sync.dma_start(
                w_l2[:], lin2w[:].rearrange("(k p) c -> p k c", p=P))

            ident = cp.tile([P, P], F32)
            nc.sync.dma_start(ident[:], ident_in[:])
            tbl = cp.tile([N, 168], I32)
            nc.sync.dma_start(tbl[:], tbl_in[:])
            msk = cp.tile([N, 400], I32)
            nc.sync.dma_start(msk[:], msk_in[:])
            cst = cp.tile([N, 3 * D + 2], F32)
            nc.sync.dma_start(cst[:], cst_in[:])

            # ---------------- preprocess: build HWC pooled maps ----------
            _preprocess(nc, tc, maps, img, ident)

            # ---------------- iterations (pools reuse preprocess SBUF) ----
            with tc.tile_pool(name="it", bufs=1) as sp, \
                 tc.tile_pool(name="gat", bufs=3) as gp:
                _iterations(nc, tc, sp, gp, pp, pq, ccp, maps, bnd_in, tbl,
                            msk, cst, ident, w_qkv, w_l1, w_l2,
                            traj, dbg_tok, dbg_x3, dbg_off)
    nc.finalize()
    return nc


def _preprocess(nc, tc, maps, img, ident):
    """CHW -> cell-major HWC bf16 maps via XBAR DMA transpose (no PE)."""
    maps_t = maps  # [NCELL, CH] DRAM tile

    def store_blocks(T_ap, nblk, base, cc):
        dst = bass.AP(tensor=maps_t.tensor, offset=base * CH + cc * P,
                      ap=[[CH, P], [P * CH, nblk], [1, P]])
        nc.scalar.dma_start(dst, T_ap)

    def store_rem(T_ap, rem, base, cc):
        dst = bass.AP(tensor=maps_t.tensor, offset=base * CH + cc * P,
                      ap=[[CH, rem], [1, P]])
        nc.scalar.dma_start(dst, T_ap)

    with tc.tile_pool(name="ppin", bufs=3) as pin, \
         tc.tile_pool(name="ppst", bufs=3) as pst:
        for hg in range(28):
            for cc in range(2):
                A = pin.tile([P, 8 * W0], F32, tag="A")
                nc.sync.dma_start(
                    A[:], img[P * cc:P * (cc + 1), 8 * hg:8 * hg + 8, :])
                A3 = A[:].rearrange("p (h w) -> p h w", w=W0)
                A1 = pin.tile([P, 448], F32, tag="A1")
                A13 = A1[:].rearrange("p (h w) -> p h w", w=112)
                nc.vector.tensor_tensor(out=A13, in0=A3[:, 0::2, 0::2],
                                        in1=A3[:, 0::2, 1::2], op=OP.add)
                nc.vector.tensor_tensor(out=A13, in0=A13,
                                        in1=A3[:, 1::2, 0::2], op=OP.add)
                nc.vector.tensor_tensor(out=A13, in0=A13,
                                        in1=A3[:, 1::2, 1::2], op=OP.add)
                nc.vector.tensor_scalar(out=A13, in0=A13, scalar1=0.25,
                                        scalar2=None, op0=OP.mult)
                A2 = pin.tile([P, 112], F32, tag="A2")
                A23 = A2[:].rearrange("p (h w) -> p h w", w=56)
                nc.vector.tensor_tensor(out=A23, in0=A13[:, 0::2, 0::2],
                                        in1=A13[:, 0::2, 1::2], op=OP.add)
                nc.vector.tensor_tensor(out=A23, in0=A23,
                                        in1=A13[:, 1::2, 0::2], op=OP.add)
                nc.vector.tensor_tensor(out=A23, in0=A23,
                                        in1=A13[:, 1::2, 1::2], op=OP.add)
                nc.vector.tensor_scalar(out=A23, in0=A23, scalar1=0.25,
                                        scalar2=None, op0=OP.mult)
                A4 = pin.tile([P, 28], F32, tag="A4")
                A43 = A4[:].rearrange("p (h w) -> p h w", w=28)
                nc.vector.tensor_tensor(out=A43, in0=A23[:, 0::2, 0::2],
                                        in1=A23[:, 0::2, 1::2], op=OP.add)
                nc.vector.tensor_tensor(out=A43, in0=A43,
                                        in1=A23[:, 1::2, 0::2], op=OP.add)
                nc.vector.tensor_tensor(out=A43, in0=A43,
                                        in1=A23[:, 1::2, 1::2], op=OP.add)
                nc.vector.tensor_scalar(out=A43, in0=A43, scalar1=0.25,
                                        scalar2=None, op0=OP.mult)

                # cast to bf16 (DVE; scalar ring reserved for DMA issue)
                Ab = pin.tile([P, 1792], BF16, tag="Ab")
                nc.vector.tensor_copy(out=Ab[:], in_=A[:])
                A1b = pin.tile([P, 512], BF16, tag="A1b")
                nc.vector.tensor_copy(out=A1b[:, 0:448], in_=A1[:])
                A2b = pin.tile([P, 128], BF16, tag="A2b")
                nc.vector.tensor_copy(out=A2b[:, 0:112], in_=A2[:])
                A4b = pin.tile([P, 128], BF16, tag="A4b")
                nc.vector.tensor_copy(out=A4b[:, 0:28], in_=A4[:])

                # XBAR transposes on the scalar HWDGE ring:
                # out[p, k, c] = in[c, k*128 + p]
                T0 = pst.tile([P, 14, P], BF16, tag="T0")
                nc.scalar.dma_start_transpose(T0[:], Ab[:])
                T1 = pst.tile([P, 4, P], BF16, tag="T1")
                nc.scalar.dma_start_transpose(T1[:], A1b[:])
                T2 = pst.tile([P, P], BF16, tag="T2")
                nc.scalar.dma_start_transpose(T2[:], A2b[:])
                T4 = pst.tile([P, P], BF16, tag="T4")
                nc.scalar.dma_start_transpose(T4[:], A4b[:])

                store_blocks(T0[:], 14, hg * 1792, cc)
                store_blocks(T1[:, 0:3, :], 3, 50176 + hg * 448, cc)
                store_rem(T1[0:64, 3, :], 64, 50176 + hg * 448 + 384, cc)
                store_rem(T2[0:112, :], 112, 62720 + hg * 112, cc)
                store_rem(T4[0:28, :], 28, 65856 + hg * 28, cc)


def _iterations(nc, tc, sp, gp, pp, pq, ccp, maps, bnd_in, tbl, msk, cst,
                ident, w_qkv, w_l1, w_l2, traj, dbg_tok, dbg_x3, dbg_off):
    maps_flat = maps[:].rearrange("c e -> (c e)").rearrange(
        "(a b) -> a b", b=1)
    pe_ap = cst[:, 0:D]
    fcw0 = cst[:, D:2 * D]
    fcw1 = cst[:, 2 * D:3 * D]
    fcb0 = cst[:, 3 * D:3 * D + 1]
    fcb1 = cst[:, 3 * D + 1:3 * D + 2]
    inv_sqrt_d = 1.0 / float(np.sqrt(D))

    # persistent tiles (padded regions initialized once)
    bnd = sp.tile([N, 2], I32, tag="bnd")
    nc.sync.dma_start(bnd[:], bnd_in[:])
    tok = sp.tile([N, DP], F32, tag="tok")
    x2 = sp.tile([N, DP], F32, tag="x2")
    q_t = sp.tile([N, QSP], F32, tag="q_t")
    k_t = sp.tile([N, QSP], F32, tag="k_t")
    v_t = sp.tile([N, QS], F32, tag="v_t")
    h = sp.tile([N, FSP], F32, tag="h")
    for t, c in ((tok, D), (h, FS)):
        nc.vector.memset(t[:], 0.0)
        nc.vector.memset(t[:, c:c + 1], 1.0)
    nc.vector.memset(q_t[:], 0.0)   # cols 320..384 stay zero
    nc.vector.memset(k_t[:], 0.0)

    for it in range(NITER):
        # ---- gather indices [N, 4, 7] ----
        bsh = sp.tile([N, 8], I32, tag="bsh")
        nc.vector.tensor_tensor(
            out=bsh[:].rearrange("n (a s) -> n a s", a=2),
            in0=_bc(bnd[:].rearrange("n (a s) -> n a s", s=1), [N, 2, 4]),
            in1=_bc(tbl[:, 140:144].rearrange("n (a s) -> n a s", a=1),
                    [N, 2, 4]),
            op=OP.arith_shift_right)
        bx7 = _bc(bsh[:, 0:4].rearrange("n (s a) -> n s a", a=1), [N, 4, 7])
        by7 = _bc(bsh[:, 4:8].rearrange("n (s a) -> n s a", a=1), [N, 4, 7])
        idx = sp.tile([N, 28], I32, tag="idx")
        idx3 = idx[:].rearrange("n (s d) -> n s d", s=4)
        tbl3 = tbl[:].rearrange("n (g c) -> n g c", c=28)
        nc.vector.tensor_tensor(
            out=idx3, in0=bx7,
            in1=tbl3[:, 0, :].rearrange("n (s d) -> n s d", s=4), op=OP.add)
        nc.vector.tensor_scalar(out=idx[:], in0=idx[:], scalar1=0,
                                scalar2=None, op0=OP.max)
        nc.vector.tensor_tensor(
            out=idx3, in0=idx3,
            in1=tbl3[:, 1, :].rearrange("n (s d) -> n s d", s=4), op=OP.min)
        nc.vector.tensor_tensor(
            out=idx3, in0=idx3,
            in1=tbl3[:, 2, :].rearrange("n (s d) -> n s d", s=4), op=OP.mult)
        nc.vector.tensor_tensor(out=idx3, in0=idx3, in1=by7, op=OP.add)
        nc.vector.tensor_scalar(out=idx[:], in0=idx[:], scalar1=-3,
                                scalar2=0, op0=OP.add, op1=OP.max)
        nc.vector.tensor_tensor(
            out=idx3, in0=idx3,
            in1=tbl3[:, 4, :].rearrange("n (s d) -> n s d", s=4), op=OP.min)
        nc.vector.tensor_tensor(
            out=idx3, in0=idx3,
            in1=tbl3[:, 3, :].rearrange("n (s d) -> n s d", s=4), op=OP.add)
        nc.vector.tensor_scalar(out=idx[:], in0=idx[:], scalar1=CH,
                                scalar2=None, op0=OP.mult)

        # ---- masks [N, 196] ----
        bx49 = _bc(bsh[:, 0:4].rearrange("n (s a) -> n s a", a=1), [N, 4, 49])
        by49 = _bc(bsh[:, 4:8].rearrange("n (s a) -> n s a", a=1), [N, 4, 49])
        m3 = lambda t: t.rearrange("n (s d) -> n s d", s=4)
        mi = sp.tile([N, 196], I32, tag="mi")
        mt = sp.tile([N, 196], I32, tag="mt")
        hs49 = _bc(msk[:, 392:396].rearrange("n (s a) -> n s a", a=1),
                   [N, 4, 49])
        ws49 = _bc(msk[:, 396:400].rearrange("n (s a) -> n s a", a=1),
                   [N, 4, 49])
        nc.vector.tensor_tensor(out=m3(mt[:]), in0=bx49,
                                in1=m3(msk[:, 0:196]), op=OP.add)
        nc.vector.tensor_tensor(out=m3(mi[:]), in0=m3(mt[:]), in1=hs49,
                                op=OP.is_le)
        nc.vector.tensor_scalar(out=mt[:], in0=mt[:], scalar1=0,
                                scalar2=None, op0=OP.is_ge)
        nc.vector.tensor_tensor(out=mi[:], in0=mi[:], in1=mt[:],
                                op=OP.bitwise_and)
        nc.vector.tensor_tensor(out=m3(mt[:]), in0=by49,
                                in1=m3(msk[:, 196:392]), op=OP.add)
        mw = sp.tile([N, 196], I32, tag="mw")
        nc.vector.tensor_tensor(out=m3(mw[:]), in0=m3(mt[:]), in1=ws49,
                                op=OP.is_le)
        nc.vector.tensor_tensor(out=mi[:], in0=mi[:], in1=mw[:],
                                op=OP.bitwise_and)
        nc.vector.tensor_scalar(out=mt[:], in0=mt[:], scalar1=0,
                                scalar2=None, op0=OP.is_ge)
        nc.vector.tensor_tensor(out=mi[:], in0=mi[:], in1=mt[:],
                                op=OP.bitwise_and)
        mask = sp.tile([N, 196], F32, tag="mask")
        nc.vector.tensor_copy(out=mask[:], in_=mi[:])

        # ---- gathers + dots (per 7-cell run; qf run gathered first) ----
        dots = sp.tile([N, 196], F32, tag="dots")
        qf = sp.tile([N, CH], F32, tag="qf")
        run_order = [(0, 3), (0, 0), (0, 1), (0, 2), (0, 4), (0, 5), (0, 6)]
        run_order += [(s, dx) for s in range(1, 4) for dx in range(7)]
        qfb = sp.tile([N, CH], BF16, tag="qfb")
        for (s, dx) in run_order:
            K = gp.tile([N, 7 * CH], BF16, tag="K")
            nc.gpsimd.indirect_dma_start(
                out=K[:], out_offset=None, in_=maps_flat,
                in_offset=bass.IndirectOffsetOnAxis(
                    ap=idx[:, s * 7 + dx:s * 7 + dx + 1], axis=0))
            K3 = K[:].rearrange("n (d c) -> n d c", c=CH)
            if s == 0 and dx == 3:
                nc.vector.tensor_copy(out=qf[:], in_=K3[:, 3, :])
                nc.vector.tensor_copy(out=qfb[:], in_=K3[:, 3, :])
            Kf = sp.tile([N, 7 * CH], F32, tag="Kf")
            Kf3 = Kf[:].rearrange("n (d c) -> n d c", c=CH)
            nc.vector.tensor_tensor(
                out=Kf3, in0=K3,
                in1=_bc(qfb[:].rearrange("n (a c) -> n a c", a=1),
                        [N, 7, CH]),
                op=OP.mult)
            j = s * 49 + dx * 7
            nc.vector.tensor_reduce(
                out=dots[:, j:j + 7].rearrange("n (d a) -> n d a", a=1),
                in_=Kf3, op=OP.add, axis=AX.X)
        nc.vector.tensor_tensor(out=dots[:], in0=dots[:], in1=mask[:],
                                op=OP.mult)

        # ---- AllGather (qf | dots) ----
        agi = sp.tile([N, CH + 196], F32, tag="agi")
        nc.vector.tensor_copy(out=agi[:, 0:CH], in_=qf[:])
        nc.vector.tensor_copy(out=agi[:, CH:], in_=dots[:])
        cin = ccp.tile([N, CH + 196], F32, tag="cin")
        cout = ccp.tile([4 * N, CH + 196], F32, tag="cout")
        nc.sync.dma_start(cin[:], agi[:])
        nc.gpsimd.collective_compute(
            "AllGather", OP.bypass, ins=[cin[:]], outs=[cout[:]],
            replica_groups=GROUPS)

        # ---- tokens ----
        for r in range(4):
            nc.sync.dma_start(tok[:, CH * r:CH * (r + 1)],
                              cout[N * r:N * (r + 1), 0:CH])
        dsum = sp.tile([N, 4, 196], F32, tag="dsum")
        nc.sync.dma_start(
            dsum[:], cout[:].rearrange("(r n) e -> r n e", n=N)
            .rearrange("r n e -> n r e")[:, :, CH:])
        nc.vector.tensor_tensor(out=dsum[:, 0, :], in0=dsum[:, 0, :],
                                in1=dsum[:, 1, :], op=OP.add)
        nc.vector.tensor_tensor(out=dsum[:, 2, :], in0=dsum[:, 2, :],
                                in1=dsum[:, 3, :], op=OP.add)
        nc.vector.tensor_tensor(out=tok[:, 1024:1220], in0=dsum[:, 0, :],
                                in1=dsum[:, 2, :], op=OP.add)
        nc.vector.tensor_copy(out=tok[:, 1220:1222], in_=bnd[:])
        _ln(nc, sp, tok[:, 0:D], D, "l1")
        nc.vector.tensor_tensor(out=tok[:, 0:D], in0=tok[:, 0:D],
                                in1=pe_ap, op=OP.add)
        if it == 0:
            nc.sync.dma_start(dbg_tok[:], tok[:, 0:D])

        # ---- QKV slices (out_proj folded into V), resident weights ----
        xt = _tp_cols(nc, sp, pq, tok[:], DP, ident, "xt")
        for ci, dst in ((0, q_t), (1, k_t), (2, v_t)):
            ps = pp.tile([N, QS], F32, tag="mmps", space="PSUM")
            for k in range(10):
                nc.tensor.matmul(ps[:], xt[:, k, :],
                                 w_qkv[:, k, QS * ci:QS * (ci + 1)],
                                 start=(k == 0), stop=(k == 9))
            nc.vector.tensor_copy(out=dst[:, 0:QS], in_=ps[:])

        # ---- attention scores: partial over this core's 320 cols ----
        qT = _tp_cols(nc, sp, pq, q_t[:], QSP, ident, "qT")
        kT = _tp_cols(nc, sp, pq, k_t[:], QSP, ident, "kT")
        sc_ps = pp.tile([N, N], F32, tag="mmps", space="PSUM")
        for k in range(3):
            nc.tensor.matmul(sc_ps[:], qT[:, k, :], kT[:, k, :],
                             start=(k == 0), stop=(k == 2))
        sc = sp.tile([N, N], F32, tag="sc")
        nc.vector.tensor_scalar(out=sc[:], in0=sc_ps[:], scalar1=inv_sqrt_d,
                                scalar2=None, op0=OP.mult)
        sin_ = ccp.tile([N, N], F32, tag="sin")
        sout = ccp.tile([4 * N, N], F32, tag="sout")
        nc.sync.dma_start(sin_[:], sc[:])
        nc.gpsimd.collective_compute(
            "AllGather", OP.bypass, ins=[sin_[:]], outs=[sout[:]],
            replica_groups=GROUPS)
        scg = sp.tile([N, 4, N], F32, tag="scg")
        nc.sync.dma_start(
            scg[:], sout[:].rearrange("(r n) e -> r n e", n=N)
            .rearrange("r n e -> n r e"))
        nc.vector.tensor_tensor(out=scg[:, 0, :], in0=scg[:, 0, :],
                                in1=scg[:, 1, :], op=OP.add)
        nc.vector.tensor_tensor(out=scg[:, 2, :], in0=scg[:, 2, :],
                                in1=scg[:, 3, :], op=OP.add)
        nc.vector.tensor_tensor(out=sc[:], in0=scg[:, 0, :],
                                in1=scg[:, 2, :], op=OP.add)

        # ---- softmax (replicated) ----
        mx = sp.tile([N, 1], F32, tag="mx")
        nc.vector.tensor_reduce(out=mx[:], in_=sc[:], op=OP.max, axis=AX.X)
        nmx = sp.tile([N, 1], F32, tag="nmx")
        nc.vector.tensor_scalar(out=nmx[:], in0=mx[:], scalar1=-1.0,
                                scalar2=None, op0=OP.mult)
        esum = sp.tile([N, 1], F32, tag="esum")
        nc.scalar.activation(out=sc[:], in_=sc[:], func=AF.Exp,
                             bias=nmx[:], accum_out=esum[:])
        rsum = sp.tile([N, 1], F32, tag="rsum")
        nc.vector.reciprocal(out=rsum[:], in_=esum[:])
        nc.vector.tensor_scalar(out=sc[:], in0=sc[:], scalar1=rsum[:],
                                scalar2=None, op0=OP.mult)
        smT_ps = pq.tile([N, N], F32, tag="tpps", space="PSUM")
        nc.tensor.transpose(out=smT_ps[:], in_=sc[:], identity=ident[:N, :N])
        smT = sp.tile([N, N], F32, tag="smT")
        nc.vector.tensor_copy(out=smT[:], in_=smT_ps[:])

        # ---- attention output slice [N, 320] + AllGather ----
        at_ps = pp.tile([N, QS], F32, tag="mmps", space="PSUM")
        nc.tensor.matmul(at_ps[:], smT[:], v_t[:], start=True, stop=True)
        at_s = sp.tile([N, QS], F32, tag="at_s")
        nc.vector.tensor_copy(out=at_s[:], in_=at_ps[:])
        ain = ccp.tile([N, QS], F32, tag="ain")
        aout = ccp.tile([4 * N, QS], F32, tag="aout")
        nc.sync.dma_start(ain[:], at_s[:])
        nc.gpsimd.collective_compute(
            "AllGather", OP.bypass, ins=[ain[:]], outs=[aout[:]],
            replica_groups=GROUPS)
        for r in range(4):
            nc.sync.dma_start(x2[:, QS * r:QS * (r + 1)],
                              aout[N * r:N * (r + 1), :])
        # x2 = attn + tok over full padded width (restores bias col 1222)
        nc.vector.tensor_tensor(out=x2[:], in0=x2[:], in1=tok[:], op=OP.add)
        _ln(nc, sp, x2[:, 0:D], D, "l2")

        # ---- FF: lin1 col-slice -> relu -> lin2 row-slice (partial) ----
        x2T = _tp_cols(nc, sp, pq, x2[:], DP, ident, "x2T")
        ps1 = pp.tile([N, FS], F32, tag="mmps", space="PSUM")
        for k in range(10):
            nc.tensor.matmul(ps1[:], x2T[:, k, :], w_l1[:, k, :],
                             start=(k == 0), stop=(k == 9))
        nc.vector.tensor_scalar(out=h[:, 0:FS], in0=ps1[:],
                                scalar1=0.0, scalar2=None, op0=OP.max)
        hT = _tp_cols(nc, sp, pq, h[:], FSP, ident, "hT")
        x3 = sp.tile([N, D], F32, tag="x3")
        for ccol in range(3):
            c0 = 512 * ccol
            cw = min(512, D - c0)
            ps = pp.tile([N, 512], F32, tag="mmps", space="PSUM")
            for k in range(5):
                nc.tensor.matmul(ps[:, :cw], hT[:, k, :],
                                 w_l2[:, k, c0:c0 + cw],
                                 start=(k == 0), stop=(k == 4))
            nc.vector.tensor_copy(out=x3[:, c0:c0 + cw], in_=ps[:, :cw])
        xin = ccp.tile([N, D], F32, tag="xin")
        xout = ccp.tile([4 * N, D], F32, tag="xout")
        nc.sync.dma_start(xin[:], x3[:])
        nc.gpsimd.collective_compute(
            "AllGather", OP.bypass, ins=[xin[:]], outs=[xout[:]],
            replica_groups=GROUPS)
        xg = sp.tile([N, 2, D], F32, tag="xg")
        xo4 = xout[:].rearrange("(r n) e -> r n e", n=N) \
            .rearrange("r n e -> n r e")
        nc.sync.dma_start(xg[:], xo4[:, 0:2, :])
        nc.vector.tensor_tensor(out=x3[:], in0=xg[:, 0, :],
                                in1=xg[:, 1, :], op=OP.add)
        nc.sync.dma_start(xg[:], xo4[:, 2:4, :])
        nc.vector.tensor_tensor(out=xg[:, 0, :], in0=xg[:, 0, :],
                                in1=xg[:, 1, :], op=OP.add)
        nc.vector.tensor_tensor(out=x3[:], in0=x3[:], in1=xg[:, 0, :],
                                op=OP.add)
        nc.vector.tensor_tensor(out=x3[:], in0=x3[:], in1=x2[:, 0:D],
                                op=OP.add)
        _ln(nc, sp, x3[:], D, "l3")
        if it == 0:
            nc.sync.dma_start(dbg_x3[:], x3[:])

        # ---- fc head (only 2 outputs) ----
        f0 = sp.tile([N, D], F32, tag="lnsq")
        off = sp.tile([N, 2], F32, tag="off")
        nc.vector.tensor_tensor(out=f0[:], in0=x3[:], in1=fcw0, op=OP.mult)
        nc.vector.tensor_reduce(out=off[:, 0:1], in_=f0[:], op=OP.add,
                                axis=AX.X)
        nc.vector.tensor_tensor(out=f0[:], in0=x3[:], in1=fcw1, op=OP.mult)
        nc.vector.tensor_reduce(out=off[:, 1:2], in_=f0[:], op=OP.add,
                                axis=AX.X)
        nc.vector.tensor_tensor(out=off[:], in0=off[:],
                                in1=cst[:, 3 * D:3 * D + 2], op=OP.add)
        if it == 0:
            nc.sync.dma_start(dbg_off[:], off[:])

        # trunc toward zero: rne(off - 0.5*sign(off)); exact ints unaffected
        sgn = sp.tile([N, 2], F32, tag="sgn")
        nc.scalar.activation(out=sgn[:], in_=off[:], func=AF.Sign)
        nc.vector.tensor_scalar(out=sgn[:], in0=sgn[:], scalar1=-0.5,
                                scalar2=None, op0=OP.mult)
        nc.vector.tensor_tensor(out=off[:], in0=off[:], in1=sgn[:],
                                op=OP.add)
        ti = sp.tile([N, 2], I32, tag="ti")
        nc.vector.tensor_copy(out=ti[:], in_=off[:])
        nc.vector.tensor_tensor(out=bnd[:], in0=bnd[:], in1=ti[:], op=OP.add)
        nc.vector.tensor_scalar(out=bnd[:], in0=bnd[:], scalar1=0,
                                scalar2=223, op0=OP.max, op1=OP.min)
        nc.sync.dma_start(traj[it, :, :], bnd[:])


# ---------------------------------------------------------------------------
# host side
# ---------------------------------------------------------------------------

_NC_CACHE = {}


def _host_inputs(curr_img_features, previous_boundary, in_proj_w, in_proj_b,
                 out_proj_w, out_proj_b, lin1_w, lin1_b, lin2_w, lin2_b,
                 fc_w, fc_b):
    f32 = np.float32
    pos = np.arange(N, dtype=f32)[:, None]
    div = np.exp(np.arange(0, D, 2, dtype=f32) * (-np.log(10000.0) / D))
    pe = np.zeros((N, D), f32)
    pe[:, 0::2] = np.sin(pos * div)
    pe[:, 1::2] = np.cos(pos * div)

    Wq, Wk, Wv = (np.asarray(in_proj_w[i * D:(i + 1) * D], f32)
                  for i in range(3))
    bq, bk, bv = (np.asarray(in_proj_b[i * D:(i + 1) * D], f32)
                  for i in range(3))
    Wvp = np.asarray(out_proj_w, f32) @ Wv          # [D, D]
    bvp = np.asarray(out_proj_w, f32) @ bv + np.asarray(out_proj_b, f32)

    # padded [DP, 3*DP_cols]: rows = input dim (row D = bias), cols sharded
    qkv_full = np.zeros((DP, 3 * DP), f32)
    for i, (W, b) in enumerate(((Wq, bq), (Wk, bk), (Wvp, bvp))):
        qkv_full[0:D, DP * i:DP * i + D] = W.T
        qkv_full[D, DP * i:DP * i + D] = b

    l1 = np.zeros((DP, FF), f32)
    l1[0:D, :] = np.asarray(lin1_w, f32).T
    l1[D, :] = np.asarray(lin1_b, f32)
    l2t = np.asarray(lin2_w, f32).T                  # [FF, D]
    l2b = np.asarray(lin2_b, f32)

    cst = np.zeros((N, 3 * D + 2), f32)
    cst[:, 0:D] = pe
    cst[:, D:2 * D] = np.asarray(fc_w[:, 0, :], f32)
    cst[:, 2 * D:3 * D] = np.asarray(fc_w[:, 1, :], f32)
    cst[:, 3 * D:3 * D + 2] = np.asarray(fc_b[:, :2], f32)

    tbl = np.zeros((168,), np.int32)
    for s in range(4):
        Hs, Ws = SCALE_HW[s]
        for dx in range(7):
            j = s * 7 + dx
            tbl[j] = dx - 3
            tbl[28 + j] = Hs - 1
            tbl[56 + j] = Ws
            tbl[84 + j] = SCALE_BASE[s]
            tbl[112 + j] = Hs * Ws - 7
    tbl[140:144] = [0, 1, 2, 3]
    tblr = np.tile(tbl[None, :], (N, 1))

    mskv = np.zeros((400,), np.int32)
    for s in range(4):
        Hs, Ws = SCALE_HW[s]
        for dx in range(7):
            for dy in range(7):
                j = s * 49 + dx * 7 + dy
                mskv[j] = dx - 3
                mskv[196 + j] = dy - 3
        mskv[392 + s] = Hs - 1
        mskv[396 + s] = Ws - 1
    mskr = np.tile(mskv[None, :], (N, 1))

    ident = np.eye(P, dtype=f32)

    imgs = np.asarray(curr_img_features, f32)
    bnds = np.asarray(previous_boundary, np.int32)
    in_maps = []
    for c in range(8):
        g, q = c // 4, c % 4
        # Q/K/V col slice for this core
        qs = np.ascontiguousarray(np.concatenate(
            [qkv_full[:, DP * i + QS * q:DP * i + QS * (q + 1)]
             for i in range(3)], axis=1))           # [DP, 960]
        l1s = np.ascontiguousarray(l1[:, FS * q:FS * (q + 1)])  # [DP, 512]
        l2s = np.zeros((FSP, D), f32)
        l2s[0:FS, :] = l2t[FS * q:FS * (q + 1), :]
        if q == 0:
            l2s[FS, :] = l2b                         # bias row (once)
        m = dict(tbl_in=tblr, msk_in=mskr, cst_in=cst, ident_in=ident,
                 qkvw=qs, lin1w=l1s, lin2w=l2s)
        m["img"] = np.ascontiguousarray(imgs[g, CH * q:CH * (q + 1)])
        m["bnd_in"] = np.ascontiguousarray(bnds[g])
        in_maps.append(m)
    return in_maps


def make_in_maps(inputs):
    return _host_inputs(
        inputs["curr_img_features"], inputs["previous_boundary"],
        inputs["in_proj_w"], inputs["in_proj_b"],
        inputs["out_proj_w"], inputs["out_proj_b"],
        inputs["lin1_w"], inputs["lin1_b"],
        inputs["lin2_w"], inputs["lin2_b"],
        inputs["fc_w"], inputs["fc_b"])


def kernel(**inputs):
    from concourse.bass_utils import run_bass_kernel_spmd
    install_profile_hook()

    in_maps = make_in_maps(inputs)
    if "nc" not in _NC_CACHE:
        _NC_CACHE["nc"] = build_kernel()
    nc = _NC_CACHE["nc"]
    res = run_bass_kernel_spmd(nc, in_maps, core_ids=list(range(8)))
    kernel.last_results = res
    t0 = res.results[0]["traj"]   # batch 0
    t1 = res.results[4]["traj"]   # batch 1
    return np.stack([t0, t1], axis=1).astype(np.int32)  # [6, 2, 80, 2]


# revision 23
# speedup vs baseline: 1.2726x; 1.1632x over previous
"""Trainium2 Bass kernel for nn_NeighborModel (boundary refinement w/ sparse
neighborhood attention), SPMD over 8 NeuronCores.

Sharding: 2 groups x 4 cores; group g owns batch g; core q of a group owns a
256-channel chunk of that batch's feature maps (4 avg-pooled scales, cell-major
HWC bf16 layout, built on device via XBAR DMA transposes). Each core gathers
7x7 neighborhoods around all 80 boundary points (7-cell runs via indirect DMA),
computes partial dot-products (DVE multiply + ACT-engine accumulate), then one
AllGather per iteration exchanges (qf chunk | partial dots) in bf16. The
transformer layer (80 tokens, bf16 weights resident in SBUF) replicates Q/K
and the 80x80 score matrix on every core of a group; V' (out_proj folded in),
lin1 (512-column slice) and lin2 (matching 512-row slice) are tensor-parallel.
Per iteration: AllGather of bf16 attention-output slices and AllGather of bf16
partial-FF outputs (summed on-chip). Only the first two components of the
1026-wide fc head are computed (the rest is discarded by the model).
"""
import sys
import types
import numpy as np

import concourse.bass as bass
import concourse.bacc as bacc
import concourse.tile as tile
import concourse.mybir as mybir

P = 128
N = 80           # boundary points (tokens per batch)
D = 1222         # token dim
DP = 1280        # padded token dim (10*128); col 1222 = constant-1 bias col
FF = 2048
H0 = W0 = 224
CH = 256         # channels per core
NCELL = 66640    # cells over all scales (50176+12544+3136+784)
NITER = 6
SCALE_HW = [(224, 224), (112, 112), (56, 56), (28, 28)]
SCALE_BASE = [0, 50176, 62720, 65856]

QS = 320         # per-core col slice of V'
QKC = 2 * DP + QS   # resident qkv cols: Q full | K full | V' slice
FS = 512         # per-core col slice of lin1 / row slice of lin2
FSP = 640        # padded: col 512 = bias col (core 0 carries lin2 bias row)

F32 = mybir.dt.float32
BF16 = mybir.dt.bfloat16
I32 = mybir.dt.int32
AX = mybir.AxisListType
OP = mybir.AluOpType
AF = mybir.ActivationFunctionType

GROUPS = [[0, 1, 2, 3], [4, 5, 6, 7]]


def install_profile_hook():
    """Enable run_bass_kernel_spmd(trace=True) NTFF profiling (optional)."""
    try:
        import antenv
        if "antenv.axon_hooks" in sys.modules:
            return
        mod = types.ModuleType("antenv.axon_hooks")
        mod._hook = None
        mod.set_axon_ntff_profile_hook = lambda h: setattr(mod, "_hook", h)
        mod.get_axon_ntff_profile_hook = lambda: mod._hook
        sys.modules["antenv.axon_hooks"] = mod
        antenv.axon_hooks = mod
        from trn_agent_boot.trn_boot import _ntff_profile_via_ctypes
        mod._hook = _ntff_profile_via_ctypes("/opt/axon/libaxon_pjrt.so")
        import concourse.bass_utils as _bu
        _bu.upload_artifacts = lambda d: d
    except Exception:
        pass


# ---------------------------------------------------------------------------
# kernel build
# ---------------------------------------------------------------------------

def _bc(ap, shape):
    return ap.to_broadcast(shape)


def _ln(nc, sp, x_ap, n_feat, tag):
    """In-place LayerNorm over x_ap [N, n_feat] (gamma=1, beta=0, eps=1e-5)."""
    s = sp.tile([N, 1], F32, tag=tag + "m")
    nc.vector.tensor_reduce(out=s[:], in_=x_ap, op=OP.add, axis=AX.X)
    negm = sp.tile([N, 1], F32, tag=tag + "n")
    nc.vector.tensor_scalar(out=negm[:], in0=s[:], scalar1=-1.0 / n_feat,
                            scalar2=None, op0=OP.mult)
    sq = sp.tile([N, n_feat], F32, tag="lnsq")
    ssq = sp.tile([N, 1], F32, tag=tag + "s")
    nc.scalar.activation(out=sq[:], in_=x_ap, func=AF.Square,
                         bias=negm[:], accum_out=ssq[:])
    var = sp.tile([N, 1], F32, tag=tag + "v")
    nc.vector.tensor_scalar(out=var[:], in0=ssq[:], scalar1=1.0 / n_feat,
                            scalar2=1e-5, op0=OP.mult, op1=OP.add)
    sig = sp.tile([N, 1], F32, tag=tag + "g")
    nc.scalar.activation(out=sig[:], in_=var[:], func=AF.Sqrt)
    rstd = sp.tile([N, 1], F32, tag=tag + "r")
    nc.vector.reciprocal(out=rstd[:], in_=sig[:])
    nc.vector.tensor_scalar(out=x_ap, in0=x_ap, scalar1=negm[:],
                            scalar2=rstd[:], op0=OP.add, op1=OP.mult)


def _tp_cols(nc, sp, pq, src_ap, n_cols, idt, tag):
    """[N, n_cols] -> bf16 [128, n_cols/128, N] via PE transpose + cast."""
    nblk = n_cols // P
    dt = src_ap.dtype
    xt = sp.tile([P, nblk, N], BF16, tag=tag)
    for k in range(nblk):
        ps = pq.tile([P, N], dt, tag="tpps", space="PSUM")
        nc.tensor.transpose(out=ps[:], in_=src_ap[:, P * k:P * (k + 1)],
                            identity=idt[:N, :N])
        nc.vector.tensor_copy(out=xt[:, k, :], in_=ps[:])
    return xt


def build_kernel():
    nc = bacc.Bacc(None, target_bir_lowering=False)

    img = nc.dram_tensor("img", [CH, H0, W0], F32, kind="ExternalInput")
    bnd_in = nc.dram_tensor("bnd_in", [N, 2], I32, kind="ExternalInput")
    tbl_in = nc.dram_tensor("tbl_in", [N, 168], I32, kind="ExternalInput")
    msk_in = nc.dram_tensor("msk_in", [N, 400], I32, kind="ExternalInput")
    cst_in = nc.dram_tensor("cst_in", [N, 3 * D + 2], F32,
                            kind="ExternalInput")
    ident_in = nc.dram_tensor("ident_in", [P, P], F32, kind="ExternalInput")
    qkvw = nc.dram_tensor("qkvw", [DP, QKC], BF16, kind="ExternalInput")
    lin1w = nc.dram_tensor("lin1w", [DP, FS], BF16, kind="ExternalInput")
    lin2w = nc.dram_tensor("lin2w", [FSP, D], BF16, kind="ExternalInput")

    traj = nc.dram_tensor("traj", [NITER, N, 2], I32, kind="ExternalOutput")
    dbg_tok = nc.dram_tensor("dbg_tok", [N, D], F32, kind="ExternalOutput")
    dbg_x3 = nc.dram_tensor("dbg_x3", [N, D], F32, kind="ExternalOutput")
    dbg_off = nc.dram_tensor("dbg_off", [N, 2], F32, kind="ExternalOutput")

    with tile.TileContext(nc) as tc:
        with tc.tile_pool(name="dram", bufs=1, space="DRAM") as drp, \
             tc.tile_pool(name="cst", bufs=1) as cp, \
             tc.tile_pool(name="pp", bufs=2, space="PSUM") as pp, \
             tc.tile_pool(name="pq", bufs=2, space="PSUM") as pq, \
             tc.tile_pool(name="cc", bufs=2, space="DRAM") as ccp:

            maps = drp.tile([NCELL, CH], BF16)

            # resident bf16 weights (one big DMA each; overlap preprocess)
            w_qkv = cp.tile([P, 10, QKC], BF16)
            nc.sync.dma_start(
                w_qkv[:], qkvw[:].rearrange("(k p) c -> p k c", p=P))
            w_l1 = cp.tile([P, 10, FS], BF16)
            nc.sync.dma_start(
                w_l1[:], lin1w[:].rearrange("(k p) c -> p k c", p=P))
            w_l2 = cp.tile([P, 5, D], BF16)
            nc.sync.dma_start(
                w_l2[:], lin2w[:].rearrange("(k p) c -> p k c", p=P))

            ident = cp.tile([P, P], F32)
            nc.sync.dma_start(ident[:], ident_in[:])
            identb = cp.tile([P, P], BF16)
            nc.vector.tensor_copy(out=identb[:], in_=ident[:])
            tbl = cp.tile([N, 168], I32)
            nc.sync.dma_start(tbl[:], tbl_in[:])
            msk = cp.tile([N, 400], I32)
            nc.sync.dma_start(msk[:], msk_in[:])
            cst = cp.tile([N, 3 * D + 2], F32)
            nc.sync.dma_start(cst[:], cst_in[:])

            # ---------------- preprocess: build HWC pooled maps ----------
            _preprocess(nc, tc, maps, img)

            # ---------------- iterations (pools reuse preprocess SBUF) ----
            with tc.tile_pool(name="it", bufs=1) as sp, \
                 tc.tile_pool(name="gat", bufs=2) as gp, \
                 tc.tile_pool(name="kf", bufs=2) as kfp:
                _iterations(nc, tc, sp, gp, kfp, pp, pq, ccp, maps, bnd_in,
                            tbl, msk, cst, ident, identb, w_qkv, w_l1, w_l2,
                            traj, dbg_tok, dbg_x3, dbg_off)
    nc.finalize()
    return nc


def _preprocess(nc, tc, maps, img):
    """CHW -> cell-major HWC bf16 maps; 16-row slabs, XBAR DMA transpose.

    Per slab: pool (DVE), cast to bf16 (DVE), XBAR-transpose on the scalar
    HWDGE ring, store on the sync ring (which also carries the img loads).
    """
    maps_t = maps

    def store_blocks(T_ap, nblk, base, cc):
        dst = bass.AP(tensor=maps_t.tensor, offset=base * CH + cc * P,
                      ap=[[CH, P], [P * CH, nblk], [1, P]])
        nc.sync.dma_start(dst, T_ap)

    def store_rem(T_ap, rem, base, cc):
        dst = bass.AP(tensor=maps_t.tensor, offset=base * CH + cc * P,
                      ap=[[CH, rem], [1, P]])
        nc.sync.dma_start(dst, T_ap)

    with tc.tile_pool(name="ppin", bufs=2) as pin, \
         tc.tile_pool(name="ppst", bufs=3) as pst:
        for sb in range(14):
            for cc in range(2):
                A = pin.tile([P, 16 * W0], F32, tag="A")
                nc.sync.dma_start(
                    A[:], img[P * cc:P * (cc + 1), 16 * sb:16 * sb + 16, :])
                A3 = A[:].rearrange("p (h w) -> p h w", w=W0)
                A1 = pin.tile([P, 896], F32, tag="A1")
                A13 = A1[:].rearrange("p (h w) -> p h w", w=112)
                nc.vector.tensor_tensor(out=A13, in0=A3[:, 0::2, 0::2],
                                        in1=A3[:, 0::2, 1::2], op=OP.add)
                nc.vector.tensor_tensor(out=A13, in0=A13,
                                        in1=A3[:, 1::2, 0::2], op=OP.add)
                nc.vector.tensor_tensor(out=A13, in0=A13,
                                        in1=A3[:, 1::2, 1::2], op=OP.add)
                nc.vector.tensor_scalar(out=A13, in0=A13, scalar1=0.25,
                                        scalar2=None, op0=OP.mult)
                A2 = pin.tile([P, 224], F32, tag="A2")
                A23 = A2[:].rearrange("p (h w) -> p h w", w=56)
                nc.vector.tensor_tensor(out=A23, in0=A13[:, 0::2, 0::2],
                                        in1=A13[:, 0::2, 1::2], op=OP.add)
                nc.vector.tensor_tensor(out=A23, in0=A23,
                                        in1=A13[:, 1::2, 0::2], op=OP.add)
                nc.vector.tensor_tensor(out=A23, in0=A23,
                                        in1=A13[:, 1::2, 1::2], op=OP.add)
                nc.vector.tensor_scalar(out=A23, in0=A23, scalar1=0.25,
                                        scalar2=None, op0=OP.mult)
                A4 = pin.tile([P, 56], F32, tag="A4")
                A43 = A4[:].rearrange("p (h w) -> p h w", w=28)
                nc.vector.tensor_tensor(out=A43, in0=A23[:, 0::2, 0::2],
                                        in1=A23[:, 0::2, 1::2], op=OP.add)
                nc.vector.tensor_tensor(out=A43, in0=A43,
                                        in1=A23[:, 1::2, 0::2], op=OP.add)
                nc.vector.tensor_tensor(out=A43, in0=A43,
                                        in1=A23[:, 1::2, 1::2], op=OP.add)
                nc.vector.tensor_scalar(out=A43, in0=A43, scalar1=0.25,
                                        scalar2=None, op0=OP.mult)

                Ab = pin.tile([P, 16 * W0], BF16, tag="Ab")
                nc.vector.tensor_copy(out=Ab[:], in_=A[:])
                A1b = pin.tile([P, 896], BF16, tag="A1b")
                nc.vector.tensor_copy(out=A1b[:], in_=A1[:])
                A2b = pin.tile([P, 256], BF16, tag="A2b")
                nc.vector.tensor_copy(out=A2b[:, 0:224], in_=A2[:])
                A4b = pin.tile([P, 128], BF16, tag="A4b")
                nc.vector.tensor_copy(out=A4b[:, 0:56], in_=A4[:])

                # XBAR transposes: out[p, k, c] = in[c, k*128 + p]
                T0 = pst.tile([P, 28, P], BF16, tag="T0")
                nc.scalar.dma_start_transpose(T0[:], Ab[:])
                T1 = pst.tile([P, 7, P], BF16, tag="T1")
                nc.scalar.dma_start_transpose(T1[:], A1b[:])
                T2 = pst.tile([P, 2, P], BF16, tag="T2")
                nc.scalar.dma_start_transpose(T2[:], A2b[:])
                T4 = pst.tile([P, P], BF16, tag="T4")
                nc.scalar.dma_start_transpose(T4[:], A4b[:])

                store_blocks(T0[:], 28, sb * 3584, cc)
                store_blocks(T1[:], 7, 50176 + sb * 896, cc)
                store_blocks(T2[:, 0:1, :], 1, 62720 + sb * 224, cc)
                store_rem(T2[0:96, 1, :], 96, 62720 + sb * 224 + 128, cc)
                store_rem(T4[0:56, :], 56, 65856 + sb * 56, cc)


def _iterations(nc, tc, sp, gp, kfp, pp, pq, ccp, maps, bnd_in, tbl, msk,
                cst, ident, identb, w_qkv, w_l1, w_l2,
                traj, dbg_tok, dbg_x3, dbg_off):
    maps_flat = maps[:].rearrange("c e -> (c e)").rearrange(
        "(a b) -> a b", b=1)
    pe_ap = cst[:, 0:D]
    fcw0 = cst[:, D:2 * D]
    fcw1 = cst[:, 2 * D:3 * D]
    inv_sqrt_d = 1.0 / float(np.sqrt(D))

    # persistent tiles (padded regions initialized once)
    bnd = sp.tile([N, 2], I32, tag="bnd")
    nc.sync.dma_start(bnd[:], bnd_in[:])
    tok = sp.tile([N, DP], F32, tag="tok")
    x2 = sp.tile([N, DP], F32, tag="x2")
    attb = sp.tile([N, DP], BF16, tag="attb")
    tokb = sp.tile([N, 1024], BF16, tag="tokb")
    q_t = sp.tile([N, DP], BF16, tag="q_t")
    k_t = sp.tile([N, DP], BF16, tag="k_t")
    v_t = sp.tile([N, QS], BF16, tag="v_t")
    h = sp.tile([N, FSP], BF16, tag="h")
    nc.vector.memset(tok[:], 0.0)
    nc.vector.memset(tok[:, D:D + 1], 1.0)
    nc.vector.memset(h[:], 0.0)
    nc.vector.memset(h[:, FS:FS + 1], 1.0)

    for it in range(NITER):
        # ---- gather indices [N, 4, 7] ----
        bsh = sp.tile([N, 8], I32, tag="bsh")
        nc.vector.tensor_tensor(
            out=bsh[:].rearrange("n (a s) -> n a s", a=2),
            in0=_bc(bnd[:].rearrange("n (a s) -> n a s", s=1), [N, 2, 4]),
            in1=_bc(tbl[:, 140:144].rearrange("n (a s) -> n a s", a=1),
                    [N, 2, 4]),
            op=OP.arith_shift_right)
        bx7 = _bc(bsh[:, 0:4].rearrange("n (s a) -> n s a", a=1), [N, 4, 7])
        by7 = _bc(bsh[:, 4:8].rearrange("n (s a) -> n s a", a=1), [N, 4, 7])
        idx = sp.tile([N, 28], I32, tag="idx")
        idx3 = idx[:].rearrange("n (s d) -> n s d", s=4)
        tbl3 = tbl[:].rearrange("n (g c) -> n g c", c=28)
        nc.vector.tensor_tensor(
            out=idx3, in0=bx7,
            in1=tbl3[:, 0, :].rearrange("n (s d) -> n s d", s=4), op=OP.add)
        nc.vector.tensor_scalar(out=idx[:], in0=idx[:], scalar1=0,
                                scalar2=None, op0=OP.max)
        nc.vector.tensor_tensor(
            out=idx3, in0=idx3,
            in1=tbl3[:, 1, :].rearrange("n (s d) -> n s d", s=4), op=OP.min)
        nc.vector.tensor_tensor(
            out=idx3, in0=idx3,
            in1=tbl3[:, 2, :].rearrange("n (s d) -> n s d", s=4), op=OP.mult)
        nc.vector.tensor_tensor(out=idx3, in0=idx3, in1=by7, op=OP.add)
        nc.vector.tensor_scalar(out=idx[:], in0=idx[:], scalar1=-3,
                                scalar2=0, op0=OP.add, op1=OP.max)
        nc.vector.tensor_tensor(
            out=idx3, in0=idx3,
            in1=tbl3[:, 4, :].rearrange("n (s d) -> n s d", s=4), op=OP.min)
        nc.vector.tensor_tensor(
            out=idx3, in0=idx3,
            in1=tbl3[:, 3, :].rearrange("n (s d) -> n s d", s=4), op=OP.add)
        nc.vector.tensor_scalar(out=idx[:], in0=idx[:], scalar1=CH,
                                scalar2=None, op0=OP.mult)

        # ---- masks [N, 196] ----
        bx49 = _bc(bsh[:, 0:4].rearrange("n (s a) -> n s a", a=1), [N, 4, 49])
        by49 = _bc(bsh[:, 4:8].rearrange("n (s a) -> n s a", a=1), [N, 4, 49])
        m3 = lambda t: t.rearrange("n (s d) -> n s d", s=4)
        mi = sp.tile([N, 196], I32, tag="mi")
        mt = sp.tile([N, 196], I32, tag="mt")
        hs49 = _bc(msk[:, 392:396].rearrange("n (s a) -> n s a", a=1),
                   [N, 4, 49])
        ws49 = _bc(msk[:, 396:400].rearrange("n (s a) -> n s a", a=1),
                   [N, 4, 49])
        nc.vector.tensor_tensor(out=m3(mt[:]), in0=bx49,
                                in1=m3(msk[:, 0:196]), op=OP.add)
        nc.vector.tensor_tensor(out=m3(mi[:]), in0=m3(mt[:]), in1=hs49,
                                op=OP.is_le)
        nc.vector.tensor_scalar(out=mt[:], in0=mt[:], scalar1=0,
                                scalar2=None, op0=OP.is_ge)
        nc.vector.tensor_tensor(out=mi[:], in0=mi[:], in1=mt[:],
                                op=OP.bitwise_and)
        nc.vector.tensor_tensor(out=m3(mt[:]), in0=by49,
                                in1=m3(msk[:, 196:392]), op=OP.add)
        mw = sp.tile([N, 196], I32, tag="mw")
        nc.vector.tensor_tensor(out=m3(mw[:]), in0=m3(mt[:]), in1=ws49,
                                op=OP.is_le)
        nc.vector.tensor_tensor(out=mi[:], in0=mi[:], in1=mw[:],
                                op=OP.bitwise_and)
        nc.vector.tensor_scalar(out=mt[:], in0=mt[:], scalar1=0,
                                scalar2=None, op0=OP.is_ge)
        nc.vector.tensor_tensor(out=mi[:], in0=mi[:], in1=mt[:],
                                op=OP.bitwise_and)
        mask = sp.tile([N, 196], F32, tag="mask")
        nc.vector.tensor_copy(out=mask[:], in_=mi[:])

        # ---- gathers + dots: DVE multiply, ACT-engine accumulate ----
        dots = sp.tile([N, 196], F32, tag="dots")
        qf = sp.tile([N, CH], F32, tag="qf")
        qfb = sp.tile([N, CH], BF16, tag="qfb")
        run_order = [(0, 3), (0, 0), (0, 1), (0, 2), (0, 4), (0, 5), (0, 6)]
        run_order += [(s, dx) for s in range(1, 4) for dx in range(7)]
        for (s, dx) in run_order:
            K = gp.tile([N, 7 * CH], BF16, tag="K")
            nc.gpsimd.indirect_dma_start(
                out=K[:], out_offset=None, in_=maps_flat,
                in_offset=bass.IndirectOffsetOnAxis(
                    ap=idx[:, s * 7 + dx:s * 7 + dx + 1], axis=0))
            K3 = K[:].rearrange("n (d c) -> n d c", c=CH)
            if s == 0 and dx == 3:
                nc.vector.tensor_copy(out=qf[:], in_=K3[:, 3, :])
                nc.vector.tensor_copy(out=qfb[:], in_=K3[:, 3, :])
            Kf = kfp.tile([N, 7 * CH], BF16, tag="Kf")
            Kf3 = Kf[:].rearrange("n (d c) -> n d c", c=CH)
            nc.vector.tensor_tensor(
                out=Kf3, in0=K3,
                in1=_bc(qfb[:].rearrange("n (a c) -> n a c", a=1),
                        [N, 7, CH]),
                op=OP.mult)
            j = s * 49 + dx * 7
            for d2 in range(7):
                nc.scalar.activation(
                    out=Kf3[:, d2, :], in_=Kf3[:, d2, :], func=AF.Copy,
                    accum_out=dots[:, j + d2:j + d2 + 1])
        nc.vector.tensor_tensor(out=dots[:], in0=dots[:], in1=mask[:],
                                op=OP.mult)

        # ---- AllGather (qf | dots), bf16 payload ----
        agi = sp.tile([N, CH + 196], BF16, tag="agi")
        nc.vector.tensor_copy(out=agi[:, 0:CH], in_=qfb[:])
        nc.vector.tensor_copy(out=agi[:, CH:], in_=dots[:])
        cin = ccp.tile([N, CH + 196], BF16, tag="cin")
        cout = ccp.tile([4 * N, CH + 196], BF16, tag="cout")
        nc.sync.dma_start(cin[:], agi[:])
        nc.gpsimd.collective_compute(
            "AllGather", OP.bypass, ins=[cin[:]], outs=[cout[:]],
            replica_groups=GROUPS)

        # ---- tokens ----
        for r in range(4):
            nc.sync.dma_start(tokb[:, CH * r:CH * (r + 1)],
                              cout[N * r:N * (r + 1), 0:CH])
        nc.vector.tensor_copy(out=tok[:, 0:1024], in_=tokb[:])
        dsum = sp.tile([N, 4, 196], BF16, tag="dsum")
        nc.sync.dma_start(
            dsum[:], cout[:].rearrange("(r n) e -> r n e", n=N)
            .rearrange("r n e -> n r e")[:, :, CH:])
        dsf = sp.tile([N, 2, 196], F32, tag="dsf")
        nc.vector.tensor_tensor(out=dsf[:, 0, :], in0=dsum[:, 0, :],
                                in1=dsum[:, 1, :], op=OP.add)
        nc.vector.tensor_tensor(out=dsf[:, 1, :], in0=dsum[:, 2, :],
                                in1=dsum[:, 3, :], op=OP.add)
        nc.vector.tensor_tensor(out=tok[:, 1024:1220], in0=dsf[:, 0, :],
                                in1=dsf[:, 1, :], op=OP.add)
        nc.vector.tensor_copy(out=tok[:, 1220:1222], in_=bnd[:])
        _ln(nc, sp, tok[:, 0:D], D, "l1")
        nc.vector.tensor_tensor(out=tok[:, 0:D], in0=tok[:, 0:D],
                                in1=pe_ap, op=OP.add)
        if it == 0:
            nc.sync.dma_start(dbg_tok[:], tok[:, 0:D])

        # ---- QKV: Q,K replicated full; V' 320-col slice ----
        xt = _tp_cols(nc, sp, pq, tok[:], DP, ident, "xt")
        qk_chains = [(0, 512, q_t, 0), (512, 512, q_t, 512),
                     (1024, 256, q_t, 1024),
                     (1280, 512, k_t, 0), (1792, 512, k_t, 512),
                     (2304, 256, k_t, 1024),
                     (2560, QS, v_t, 0)]
        for (c0, cw, dst, d0) in qk_chains:
            ps = pp.tile([N, 512], F32, tag="mmps", space="PSUM")
            for k in range(10):
                nc.tensor.matmul(ps[:, :cw], xt[:, k, :],
                                 w_qkv[:, k, c0:c0 + cw],
                                 start=(k == 0), stop=(k == 9))
            nc.vector.tensor_copy(out=dst[:, d0:d0 + cw], in_=ps[:, :cw])

        # ---- attention scores (replicated, no collective) ----
        qT = _tp_cols(nc, sp, pq, q_t[:], DP, identb, "qT")
        kT = _tp_cols(nc, sp, pq, k_t[:], DP, identb, "kT")
        sc_ps = pp.tile([N, N], F32, tag="mmps", space="PSUM")
        for k in range(10):
            nc.tensor.matmul(sc_ps[:], qT[:, k, :], kT[:, k, :],
                             start=(k == 0), stop=(k == 9))
        sc = sp.tile([N, N], F32, tag="sc")
        nc.vector.tensor_scalar(out=sc[:], in0=sc_ps[:], scalar1=inv_sqrt_d,
                                scalar2=None, op0=OP.mult)

        # ---- softmax (replicated) ----
        mx = sp.tile([N, 1], F32, tag="mx")
        nc.vector.tensor_reduce(out=mx[:], in_=sc[:], op=OP.max, axis=AX.X)
        nmx = sp.tile([N, 1], F32, tag="nmx")
        nc.vector.tensor_scalar(out=nmx[:], in0=mx[:], scalar1=-1.0,
                                scalar2=None, op0=OP.mult)
        esum = sp.tile([N, 1], F32, tag="esum")
        nc.scalar.activation(out=sc[:], in_=sc[:], func=AF.Exp,
                             bias=nmx[:], accum_out=esum[:])
        rsum = sp.tile([N, 1], F32, tag="rsum")
        nc.vector.reciprocal(out=rsum[:], in_=esum[:])
        nc.vector.tensor_scalar(out=sc[:], in0=sc[:], scalar1=rsum[:],
                                scalar2=None, op0=OP.mult)
        smT_ps = pq.tile([N, N], F32, tag="tpps", space="PSUM")
        nc.tensor.transpose(out=smT_ps[:], in_=sc[:], identity=ident[:N, :N])
        smTb = sp.tile([N, N], BF16, tag="smTb")
        nc.vector.tensor_copy(out=smTb[:], in_=smT_ps[:])

        # ---- attention output slice [N, 320] + AllGather (bf16) ----
        at_ps = pp.tile([N, QS], F32, tag="mmps", space="PSUM")
        nc.tensor.matmul(at_ps[:], smTb[:], v_t[:], start=True, stop=True)
        at_s = sp.tile([N, QS], BF16, tag="at_s")
        nc.vector.tensor_copy(out=at_s[:], in_=at_ps[:])
        ain = ccp.tile([N, QS], BF16, tag="ain")
        aout = ccp.tile([4 * N, QS], BF16, tag="aout")
        nc.sync.dma_start(ain[:], at_s[:])
        nc.gpsimd.collective_compute(
            "AllGather", OP.bypass, ins=[ain[:]], outs=[aout[:]],
            replica_groups=GROUPS)
        for r in range(4):
            nc.sync.dma_start(attb[:, QS * r:QS * (r + 1)],
                              aout[N * r:N * (r + 1), :])
        # x2 = attn + tok over full padded width (restores bias col 1222)
        nc.vector.tensor_copy(out=x2[:], in_=attb[:])
        nc.vector.tensor_tensor(out=x2[:], in0=x2[:], in1=tok[:], op=OP.add)
        _ln(nc, sp, x2[:, 0:D], D, "l2")

        # ---- FF: lin1 col-slice -> relu -> lin2 row-slice (partial) ----
        x2T = _tp_cols(nc, sp, pq, x2[:], DP, ident, "x2T")
        ps1 = pp.tile([N, FS], F32, tag="mmps", space="PSUM")
        for k in range(10):
            nc.tensor.matmul(ps1[:], x2T[:, k, :], w_l1[:, k, :],
                             start=(k == 0), stop=(k == 9))
        nc.vector.tensor_scalar(out=h[:, 0:FS], in0=ps1[:],
                                scalar1=0.0, scalar2=None, op0=OP.max)
        hT = _tp_cols(nc, sp, pq, h[:], FSP, identb, "hT")
        x3p = sp.tile([N, D], BF16, tag="x3p")
        for ccol in range(3):
            c0 = 512 * ccol
            cw = min(512, D - c0)
            ps = pp.tile([N, 512], F32, tag="mmps", space="PSUM")
            for k in range(5):
                nc.tensor.matmul(ps[:, :cw], hT[:, k, :],
                                 w_l2[:, k, c0:c0 + cw],
                                 start=(k == 0), stop=(k == 4))
            nc.vector.tensor_copy(out=x3p[:, c0:c0 + cw], in_=ps[:, :cw])
        xin = ccp.tile([N, D], BF16, tag="xin")
        xout = ccp.tile([4 * N, D], BF16, tag="xout")
        nc.sync.dma_start(xin[:], x3p[:])
        nc.gpsimd.collective_compute(
            "AllGather", OP.bypass, ins=[xin[:]], outs=[xout[:]],
            replica_groups=GROUPS)
        x3 = sp.tile([N, D], F32, tag="x3")
        xg = sp.tile([N, 2, D], BF16, tag="xg")
        xo4 = xout[:].rearrange("(r n) e -> r n e", n=N) \
            .rearrange("r n e -> n r e")
        nc.sync.dma_start(xg[:], xo4[:, 0:2, :])
        nc.vector.tensor_tensor(out=x3[:], in0=xg[:, 0, :],
                                in1=xg[:, 1, :], op=OP.add)
        nc.sync.dma_start(xg[:], xo4[:, 2:4, :])
        xf = sp.tile([N, D], F32, tag="lnsq")
        nc.vector.tensor_tensor(out=xf[:], in0=xg[:, 0, :],
                                in1=xg[:, 1, :], op=OP.add)
        nc.vector.tensor_tensor(out=x3[:], in0=x3[:], in1=xf[:], op=OP.add)
        nc.vector.tensor_tensor(out=x3[:], in0=x3[:], in1=x2[:, 0:D],
                                op=OP.add)
        _ln(nc, sp, x3[:], D, "l3")
        if it == 0:
            nc.sync.dma_start(dbg_x3[:], x3[:])

        # ---- fc head (only 2 outputs): DVE multiply, ACT accumulate ----
        f0 = sp.tile([N, D], F32, tag="lnsq")
        off = sp.tile([N, 2], F32, tag="off")
        nc.vector.tensor_tensor(out=f0[:], in0=x3[:], in1=fcw0, op=OP.mult)
        nc.scalar.activation(out=f0[:], in_=f0[:], func=AF.Copy,
                             accum_out=off[:, 0:1])
        f1 = sp.tile([N, D], F32, tag="f1")
        nc.vector.tensor_tensor(out=f1[:], in0=x3[:], in1=fcw1, op=OP.mult)
        nc.scalar.activation(out=f1[:], in_=f1[:], func=AF.Copy,
                             accum_out=off[:, 1:2])
        nc.vector.tensor_tensor(out=off[:], in0=off[:],
                                in1=cst[:, 3 * D:3 * D + 2], op=OP.add)
        if it == 0:
            nc.sync.dma_start(dbg_off[:], off[:])

        # trunc toward zero: rne(off - 0.5*sign(off)); exact ints unaffected
        sgn = sp.tile([N, 2], F32, tag="sgn")
        nc.scalar.activation(out=sgn[:], in_=off[:], func=AF.Sign)
        nc.vector.tensor_scalar(out=sgn[:], in0=sgn[:], scalar1=-0.5,
                                scalar2=None, op0=OP.mult)
        nc.vector.tensor_tensor(out=off[:], in0=off[:], in1=sgn[:],
                                op=OP.add)
        ti = sp.tile([N, 2], I32, tag="ti")
        nc.vector.tensor_copy(out=ti[:], in_=off[:])
        nc.vector.tensor_tensor(out=bnd[:], in0=bnd[:], in1=ti[:], op=OP.add)
        nc.vector.tensor_scalar(out=bnd[:], in0=bnd[:], scalar1=0,
                                scalar2=223, op0=OP.max, op1=OP.min)
        nc.sync.dma_start(traj[it, :, :], bnd[:])


# ---------------------------------------------------------------------------
# host side
# ---------------------------------------------------------------------------

_NC_CACHE = {}


def _host_inputs(curr_img_features, previous_boundary, in_proj_w, in_proj_b,
                 out_proj_w, out_proj_b, lin1_w, lin1_b, lin2_w, lin2_b,
                 fc_w, fc_b):
    import ml_dtypes
    f32 = np.float32
    bf = ml_dtypes.bfloat16
    pos = np.arange(N, dtype=f32)[:, None]
    div = np.exp(np.arange(0, D, 2, dtype=f32) * (-np.log(10000.0) / D))
    pe = np.zeros((N, D), f32)
    pe[:, 0::2] = np.sin(pos * div)
    pe[:, 1::2] = np.cos(pos * div)

    Wq, Wk, Wv = (np.asarray(in_proj_w[i * D:(i + 1) * D], f32)
                  for i in range(3))
    bq, bk, bv = (np.asarray(in_proj_b[i * D:(i + 1) * D], f32)
                  for i in range(3))
    Wvp = np.asarray(out_proj_w, f32) @ Wv          # [D, D]
    bvp = np.asarray(out_proj_w, f32) @ bv + np.asarray(out_proj_b, f32)

    # padded [DP, 3*DP]: rows = input dim (row D = bias)
    qkv_full = np.zeros((DP, 3 * DP), f32)
    for i, (W, b) in enumerate(((Wq, bq), (Wk, bk), (Wvp, bvp))):
        qkv_full[0:D, DP * i:DP * i + D] = W.T
        qkv_full[D, DP * i:DP * i + D] = b

    l1 = np.zeros((DP, FF), f32)
    l1[0:D, :] = np.asarray(lin1_w, f32).T
    l1[D, :] = np.asarray(lin1_b, f32)
    l2t = np.asarray(lin2_w, f32).T                  # [FF, D]
    l2b = np.asarray(lin2_b, f32)

    cst = np.zeros((N, 3 * D + 2), f32)
    cst[:, 0:D] = pe
    cst[:, D:2 * D] = np.asarray(fc_w[:, 0, :], f32)
    cst[:, 2 * D:3 * D] = np.asarray(fc_w[:, 1, :], f32)
    cst[:, 3 * D:3 * D + 2] = np.asarray(fc_b[:, :2], f32)

    tbl = np.zeros((168,), np.int32)
    for s in range(4):
        Hs, Ws = SCALE_HW[s]
        for dx in range(7):
            j = s * 7 + dx
            tbl[j] = dx - 3
            tbl[28 + j] = Hs - 1
            tbl[56 + j] = Ws
            tbl[84 + j] = SCALE_BASE[s]
            tbl[112 + j] = Hs * Ws - 7
    tbl[140:144] = [0, 1, 2, 3]
    tblr = np.tile(tbl[None, :], (N, 1))

    mskv = np.zeros((400,), np.int32)
    for s in range(4):
        Hs, Ws = SCALE_HW[s]
        for dx in range(7):
            for dy in range(7):
                j = s * 49 + dx * 7 + dy
                mskv[j] = dx - 3
                mskv[196 + j] = dy - 3
        mskv[392 + s] = Hs - 1
        mskv[396 + s] = Ws - 1
    mskr = np.tile(mskv[None, :], (N, 1))

    ident = np.eye(P, dtype=f32)

    imgs = np.asarray(curr_img_features, f32)
    bnds = np.asarray(previous_boundary, np.int32)
    in_maps = []
    for c in range(8):
        g, q = c // 4, c % 4
        # Q full | K full | V' col slice for this core
        qs = np.ascontiguousarray(np.concatenate(
            [qkv_full[:, 0:DP], qkv_full[:, DP:2 * DP],
             qkv_full[:, 2 * DP + QS * q:2 * DP + QS * (q + 1)]],
            axis=1)).astype(bf)                      # [DP, 2880]
        l1s = np.ascontiguousarray(
            l1[:, FS * q:FS * (q + 1)]).astype(bf)   # [DP, 512]
        l2s = np.zeros((FSP, D), f32)
        l2s[0:FS, :] = l2t[FS * q:FS * (q + 1), :]
        if q == 0:
            l2s[FS, :] = l2b                         # bias row (once)
        m = dict(tbl_in=tblr, msk_in=mskr, cst_in=cst, ident_in=ident,
                 qkvw=qs, lin1w=l1s, lin2w=l2s.astype(bf))
        m["img"] = np.ascontiguousarray(imgs[g, CH * q:CH * (q + 1)])
        m["bnd_in"] = np.ascontiguousarray(bnds[g])
        in_maps.append(m)
    return in_maps


def make_in_maps(inputs):
    return _host_inputs(
        inputs["curr_img_features"], inputs["previous_boundary"],
        inputs["in_proj_w"], inputs["in_proj_b"],
        inputs["out_proj_w"], inputs["out_proj_b"],
        inputs["lin1_w"], inputs["lin1_b"],
        inputs["lin2_w"], inputs["lin2_b"],
        inputs["fc_w"], inputs["fc_b"])


def kernel(**inputs):
    from concourse.bass_utils import run_bass_kernel_spmd
    install_profile_hook()

    in_maps = make_in_maps(inputs)
    if "nc" not in _NC_CACHE:
        _NC_CACHE["nc"] = build_kernel()
    nc = _NC_CACHE["nc"]
    res = run_bass_kernel_spmd(nc, in_maps, core_ids=list(range(8)))
    kernel.last_results = res
    t0 = res.results[0]["traj"]   # batch 0
    t1 = res.results[4]["traj"]   # batch 1
    return np.stack([t0, t1], axis=1).astype(np.int32)  # [6, 2, 80, 2]


# revision 27
# speedup vs baseline: 1.2954x; 1.0179x over previous
"""Trainium2 Bass kernel for nn_NeighborModel (boundary refinement w/ sparse
neighborhood attention), SPMD over 8 NeuronCores.

Sharding: 2 groups x 4 cores; group g owns batch g; core q of a group owns a
256-channel chunk of that batch's feature maps (4 avg-pooled scales, cell-major
HWC bf16 layout, built on device via XBAR DMA transposes). Each core gathers
7x7 neighborhoods around all 80 boundary points (7-cell runs via indirect DMA),
computes partial dot-products (DVE multiply + ACT-engine accumulate), then one
AllGather per iteration exchanges (qf chunk | partial dots) in bf16. The
transformer layer (80 tokens, bf16 weights resident in SBUF) replicates Q/K
and the 80x80 score matrix on every core of a group; V' (out_proj folded in),
lin1 (512-column slice) and lin2 (matching 512-row slice) are tensor-parallel.
Per iteration: AllGather of bf16 attention-output slices and AllGather of bf16
partial-FF outputs (summed on-chip). Only the first two components of the
1026-wide fc head are computed (the rest is discarded by the model).
"""
import sys
import types
import numpy as np

import concourse.bass as bass
import concourse.bacc as bacc
import concourse.tile as tile
import concourse.mybir as mybir

P = 128
N = 80           # boundary points (tokens per batch)
D = 1222         # token dim
DP = 1280        # padded token dim (10*128); col 1222 = constant-1 bias col
FF = 2048
H0 = W0 = 224
CH = 256         # channels per core
NCELL = 66640    # cells over all scales (50176+12544+3136+784)
NITER = 6
SCALE_HW = [(224, 224), (112, 112), (56, 56), (28, 28)]
SCALE_BASE = [0, 50176, 62720, 65856]

QS = 320         # per-core col slice of V'
QKC = 2 * DP + QS   # resident qkv cols: Q full | K full | V' slice
FS = 512         # per-core col slice of lin1 / row slice of lin2
FSP = 640        # padded: col 512 = bias col (core 0 carries lin2 bias row)

F32 = mybir.dt.float32
BF16 = mybir.dt.bfloat16
I32 = mybir.dt.int32
AX = mybir.AxisListType
OP = mybir.AluOpType
AF = mybir.ActivationFunctionType

GROUPS = [[0, 1, 2, 3], [4, 5, 6, 7]]


def install_profile_hook():
    """Enable run_bass_kernel_spmd(trace=True) NTFF profiling (optional)."""
    try:
        import antenv
        if "antenv.axon_hooks" in sys.modules:
            return
        mod = types.ModuleType("antenv.axon_hooks")
        mod._hook = None
        mod.set_axon_ntff_profile_hook = lambda h: setattr(mod, "_hook", h)
        mod.get_axon_ntff_profile_hook = lambda: mod._hook
        sys.modules["antenv.axon_hooks"] = mod
        antenv.axon_hooks = mod
        from trn_agent_boot.trn_boot import _ntff_profile_via_ctypes
        mod._hook = _ntff_profile_via_ctypes("/opt/axon/libaxon_pjrt.so")
        import concourse.bass_utils as _bu
        _bu.upload_artifacts = lambda d: d
    except Exception:
        pass


# ---------------------------------------------------------------------------
# kernel build
# ---------------------------------------------------------------------------

def _bc(ap, shape):
    return ap.to_broadcast(shape)


def _ln(nc, sp, x_ap, n_feat, tag):
    """In-place LayerNorm over x_ap [N, n_feat] (gamma=1, beta=0, eps=1e-5)."""
    s = sp.tile([N, 1], F32, tag=tag + "m")
    nc.vector.tensor_reduce(out=s[:], in_=x_ap, op=OP.add, axis=AX.X)
    negm = sp.tile([N, 1], F32, tag=tag + "n")
    nc.vector.tensor_scalar(out=negm[:], in0=s[:], scalar1=-1.0 / n_feat,
                            scalar2=None, op0=OP.mult)
    sq = sp.tile([N, n_feat], F32, tag="lnsq")
    ssq = sp.tile([N, 1], F32, tag=tag + "s")
    nc.scalar.activation(out=sq[:], in_=x_ap, func=AF.Square,
                         bias=negm[:], accum_out=ssq[:])
    var = sp.tile([N, 1], F32, tag=tag + "v")
    nc.vector.tensor_scalar(out=var[:], in0=ssq[:], scalar1=1.0 / n_feat,
                            scalar2=1e-5, op0=OP.mult, op1=OP.add)
    sig = sp.tile([N, 1], F32, tag=tag + "g")
    nc.scalar.activation(out=sig[:], in_=var[:], func=AF.Sqrt)
    rstd = sp.tile([N, 1], F32, tag=tag + "r")
    nc.vector.reciprocal(out=rstd[:], in_=sig[:])
    nc.vector.tensor_scalar(out=x_ap, in0=x_ap, scalar1=negm[:],
                            scalar2=rstd[:], op0=OP.add, op1=OP.mult)


def _tp_cols(nc, sp, pq, src_ap, n_cols, idt, tag):
    """[N, n_cols] -> bf16 [128, n_cols/128, N] via PE transpose + cast."""
    nblk = n_cols // P
    dt = src_ap.dtype
    xt = sp.tile([P, nblk, N], BF16, tag=tag)
    for k in range(nblk):
        ps = pq.tile([P, N], dt, tag="tpps", space="PSUM")
        nc.tensor.transpose(out=ps[:], in_=src_ap[:, P * k:P * (k + 1)],
                            identity=idt[:N, :N])
        nc.vector.tensor_copy(out=xt[:, k, :], in_=ps[:])
    return xt


def build_kernel():
    nc = bacc.Bacc(None, target_bir_lowering=False)

    img = nc.dram_tensor("img", [CH, H0, W0], F32, kind="ExternalInput")
    bnd_in = nc.dram_tensor("bnd_in", [N, 2], I32, kind="ExternalInput")
    tbl_in = nc.dram_tensor("tbl_in", [N, 168], I32, kind="ExternalInput")
    msk_in = nc.dram_tensor("msk_in", [N, 400], I32, kind="ExternalInput")
    cst_in = nc.dram_tensor("cst_in", [N, 3 * D + 2], F32,
                            kind="ExternalInput")
    ident_in = nc.dram_tensor("ident_in", [P, P], F32, kind="ExternalInput")
    qkvw = nc.dram_tensor("qkvw", [DP, QKC], BF16, kind="ExternalInput")
    lin1w = nc.dram_tensor("lin1w", [DP, FS], BF16, kind="ExternalInput")
    lin2w = nc.dram_tensor("lin2w", [FSP, D], BF16, kind="ExternalInput")

    traj = nc.dram_tensor("traj", [NITER, N, 2], I32, kind="ExternalOutput")
    dbg_tok = nc.dram_tensor("dbg_tok", [N, D], F32, kind="ExternalOutput")
    dbg_x3 = nc.dram_tensor("dbg_x3", [N, D], F32, kind="ExternalOutput")
    dbg_off = nc.dram_tensor("dbg_off", [N, 2], F32, kind="ExternalOutput")

    with tile.TileContext(nc) as tc:
        with tc.tile_pool(name="dram", bufs=1, space="DRAM") as drp, \
             tc.tile_pool(name="cst", bufs=1) as cp, \
             tc.tile_pool(name="pp", bufs=2, space="PSUM") as pp, \
             tc.tile_pool(name="pq", bufs=2, space="PSUM") as pq, \
             tc.tile_pool(name="cc", bufs=2, space="DRAM") as ccp:

            maps = drp.tile([NCELL, CH], BF16)

            # resident bf16 weights (one big DMA each; overlap preprocess)
            w_qkv = cp.tile([P, 10, QKC], BF16)
            nc.sync.dma_start(
                w_qkv[:], qkvw[:].rearrange("(k p) c -> p k c", p=P))
            w_l1 = cp.tile([P, 10, FS], BF16)
            nc.sync.dma_start(
                w_l1[:], lin1w[:].rearrange("(k p) c -> p k c", p=P))
            w_l2 = cp.tile([P, 5, D], BF16)
            nc.sync.dma_start(
                w_l2[:], lin2w[:].rearrange("(k p) c -> p k c", p=P))

            ident = cp.tile([P, P], F32)
            nc.sync.dma_start(ident[:], ident_in[:])
            identb = cp.tile([P, P], BF16)
            nc.vector.tensor_copy(out=identb[:], in_=ident[:])
            tbl = cp.tile([N, 168], I32)
            nc.sync.dma_start(tbl[:], tbl_in[:])
            msk = cp.tile([N, 400], I32)
            nc.sync.dma_start(msk[:], msk_in[:])
            cst = cp.tile([N, 3 * D + 2], F32)
            nc.sync.dma_start(cst[:], cst_in[:])

            # ---------------- preprocess: build HWC pooled maps ----------
            _preprocess(nc, tc, maps, img)

            # ---------------- iterations (pools reuse preprocess SBUF) ----
            with tc.tile_pool(name="it", bufs=1) as sp, \
                 tc.tile_pool(name="gat", bufs=2) as gp, \
                 tc.tile_pool(name="kf", bufs=2) as kfp:
                _iterations(nc, tc, sp, gp, kfp, pp, pq, ccp, maps, bnd_in,
                            tbl, msk, cst, ident, identb, w_qkv, w_l1, w_l2,
                            traj, dbg_tok, dbg_x3, dbg_off)
    nc.finalize()
    return nc


def _preprocess(nc, tc, maps, img):
    """CHW -> cell-major HWC bf16 maps; 16-row slabs, XBAR DMA transpose.

    Per slab: pool (DVE), cast to bf16 (DVE), XBAR-transpose on the scalar
    HWDGE ring, store on the sync ring (which also carries the img loads).
    """
    maps_t = maps

    def store_blocks(T_ap, nblk, base, cc):
        dst = bass.AP(tensor=maps_t.tensor, offset=base * CH + cc * P,
                      ap=[[CH, P], [P * CH, nblk], [1, P]])
        nc.gpsimd.dma_start(dst, T_ap)

    def store_rem(T_ap, rem, base, cc):
        dst = bass.AP(tensor=maps_t.tensor, offset=base * CH + cc * P,
                      ap=[[CH, rem], [1, P]])
        nc.gpsimd.dma_start(dst, T_ap)

    with tc.tile_pool(name="ppin", bufs=2) as pin, \
         tc.tile_pool(name="ppst", bufs=3) as pst:
        for sb in range(14):
            for cc in range(2):
                A = pin.tile([P, 16 * W0], F32, tag="A")
                nc.sync.dma_start(
                    A[:], img[P * cc:P * (cc + 1), 16 * sb:16 * sb + 16, :])
                A3 = A[:].rearrange("p (h w) -> p h w", w=W0)
                A1 = pin.tile([P, 896], F32, tag="A1")
                A13 = A1[:].rearrange("p (h w) -> p h w", w=112)
                nc.vector.tensor_tensor(out=A13, in0=A3[:, 0::2, 0::2],
                                        in1=A3[:, 0::2, 1::2], op=OP.add)
                nc.vector.tensor_tensor(out=A13, in0=A13,
                                        in1=A3[:, 1::2, 0::2], op=OP.add)
                nc.vector.tensor_tensor(out=A13, in0=A13,
                                        in1=A3[:, 1::2, 1::2], op=OP.add)
                nc.vector.tensor_scalar(out=A13, in0=A13, scalar1=0.25,
                                        scalar2=None, op0=OP.mult)
                A2 = pin.tile([P, 224], F32, tag="A2")
                A23 = A2[:].rearrange("p (h w) -> p h w", w=56)
                nc.vector.tensor_tensor(out=A23, in0=A13[:, 0::2, 0::2],
                                        in1=A13[:, 0::2, 1::2], op=OP.add)
                nc.vector.tensor_tensor(out=A23, in0=A23,
                                        in1=A13[:, 1::2, 0::2], op=OP.add)
                nc.vector.tensor_tensor(out=A23, in0=A23,
                                        in1=A13[:, 1::2, 1::2], op=OP.add)
                nc.vector.tensor_scalar(out=A23, in0=A23, scalar1=0.25,
                                        scalar2=None, op0=OP.mult)
                A4 = pin.tile([P, 56], F32, tag="A4")
                A43 = A4[:].rearrange("p (h w) -> p h w", w=28)
                nc.vector.tensor_tensor(out=A43, in0=A23[:, 0::2, 0::2],
                                        in1=A23[:, 0::2, 1::2], op=OP.add)
                nc.vector.tensor_tensor(out=A43, in0=A43,
                                        in1=A23[:, 1::2, 0::2], op=OP.add)
                nc.vector.tensor_tensor(out=A43, in0=A43,
                                        in1=A23[:, 1::2, 1::2], op=OP.add)
                nc.vector.tensor_scalar(out=A43, in0=A43, scalar1=0.25,
                                        scalar2=None, op0=OP.mult)

                Ab = pin.tile([P, 16 * W0], BF16, tag="Ab")
                nc.vector.tensor_copy(out=Ab[:], in_=A[:])
                A1b = pin.tile([P, 896], BF16, tag="A1b")
                nc.scalar.activation(out=A1b[:], in_=A1[:], func=AF.Copy)
                A2b = pin.tile([P, 256], BF16, tag="A2b")
                nc.scalar.activation(out=A2b[:, 0:224], in_=A2[:],
                                     func=AF.Copy)
                A4b = pin.tile([P, 128], BF16, tag="A4b")
                nc.scalar.activation(out=A4b[:, 0:56], in_=A4[:],
                                     func=AF.Copy)

                # XBAR transposes: out[p, k, c] = in[c, k*128 + p]
                T0 = pst.tile([P, 28, P], BF16, tag="T0")
                nc.scalar.dma_start_transpose(T0[:], Ab[:])
                T1 = pst.tile([P, 7, P], BF16, tag="T1")
                nc.scalar.dma_start_transpose(T1[:], A1b[:])
                T2 = pst.tile([P, 2, P], BF16, tag="T2")
                nc.scalar.dma_start_transpose(T2[:], A2b[:])
                T4 = pst.tile([P, P], BF16, tag="T4")
                nc.scalar.dma_start_transpose(T4[:], A4b[:])

                store_blocks(T0[:], 28, sb * 3584, cc)
                store_blocks(T1[:], 7, 50176 + sb * 896, cc)
                store_blocks(T2[:, 0:1, :], 1, 62720 + sb * 224, cc)
                store_rem(T2[0:96, 1, :], 96, 62720 + sb * 224 + 128, cc)
                store_rem(T4[0:56, :], 56, 65856 + sb * 56, cc)


def _iterations(nc, tc, sp, gp, kfp, pp, pq, ccp, maps, bnd_in, tbl, msk,
                cst, ident, identb, w_qkv, w_l1, w_l2,
                traj, dbg_tok, dbg_x3, dbg_off):
    maps_flat = maps[:].rearrange("c e -> (c e)").rearrange(
        "(a b) -> a b", b=1)
    pe_ap = cst[:, 0:D]
    fcw0 = cst[:, D:2 * D]
    fcw1 = cst[:, 2 * D:3 * D]
    inv_sqrt_d = 1.0 / float(np.sqrt(D))

    # persistent tiles (padded regions initialized once)
    bnd = sp.tile([N, 2], I32, tag="bnd")
    nc.sync.dma_start(bnd[:], bnd_in[:])
    tok = sp.tile([N, DP], F32, tag="tok")
    x2 = sp.tile([N, DP], F32, tag="x2")
    attb = sp.tile([N, DP], BF16, tag="attb")
    tokb = sp.tile([N, 1024], BF16, tag="tokb")
    q_t = sp.tile([N, DP], BF16, tag="q_t")
    k_t = sp.tile([N, DP], BF16, tag="k_t")
    v_t = sp.tile([N, QS], BF16, tag="v_t")
    h = sp.tile([N, FSP], BF16, tag="h")
    nc.vector.memset(tok[:], 0.0)
    nc.vector.memset(tok[:, D:D + 1], 1.0)
    nc.vector.memset(h[:], 0.0)
    nc.vector.memset(h[:, FS:FS + 1], 1.0)

    for it in range(NITER):
        # ---- gather indices [N, 4, 7] ----
        bsh = sp.tile([N, 8], I32, tag="bsh")
        nc.vector.tensor_tensor(
            out=bsh[:].rearrange("n (a s) -> n a s", a=2),
            in0=_bc(bnd[:].rearrange("n (a s) -> n a s", s=1), [N, 2, 4]),
            in1=_bc(tbl[:, 140:144].rearrange("n (a s) -> n a s", a=1),
                    [N, 2, 4]),
            op=OP.arith_shift_right)
        bx7 = _bc(bsh[:, 0:4].rearrange("n (s a) -> n s a", a=1), [N, 4, 7])
        by7 = _bc(bsh[:, 4:8].rearrange("n (s a) -> n s a", a=1), [N, 4, 7])
        idx = sp.tile([N, 28], I32, tag="idx")
        idx3 = idx[:].rearrange("n (s d) -> n s d", s=4)
        tbl3 = tbl[:].rearrange("n (g c) -> n g c", c=28)
        nc.vector.tensor_tensor(
            out=idx3, in0=bx7,
            in1=tbl3[:, 0, :].rearrange("n (s d) -> n s d", s=4), op=OP.add)
        nc.vector.tensor_scalar(out=idx[:], in0=idx[:], scalar1=0,
                                scalar2=None, op0=OP.max)
        nc.vector.tensor_tensor(
            out=idx3, in0=idx3,
            in1=tbl3[:, 1, :].rearrange("n (s d) -> n s d", s=4), op=OP.min)
        nc.vector.tensor_tensor(
            out=idx3, in0=idx3,
            in1=tbl3[:, 2, :].rearrange("n (s d) -> n s d", s=4), op=OP.mult)
        nc.vector.tensor_tensor(out=idx3, in0=idx3, in1=by7, op=OP.add)
        nc.vector.tensor_scalar(out=idx[:], in0=idx[:], scalar1=-3,
                                scalar2=0, op0=OP.add, op1=OP.max)
        nc.vector.tensor_tensor(
            out=idx3, in0=idx3,
            in1=tbl3[:, 4, :].rearrange("n (s d) -> n s d", s=4), op=OP.min)
        nc.vector.tensor_tensor(
            out=idx3, in0=idx3,
            in1=tbl3[:, 3, :].rearrange("n (s d) -> n s d", s=4), op=OP.add)
        nc.vector.tensor_scalar(out=idx[:], in0=idx[:], scalar1=CH,
                                scalar2=None, op0=OP.mult)

        # ---- masks [N, 196] ----
        bx49 = _bc(bsh[:, 0:4].rearrange("n (s a) -> n s a", a=1), [N, 4, 49])
        by49 = _bc(bsh[:, 4:8].rearrange("n (s a) -> n s a", a=1), [N, 4, 49])
        m3 = lambda t: t.rearrange("n (s d) -> n s d", s=4)
        mi = sp.tile([N, 196], I32, tag="mi")
        mt = sp.tile([N, 196], I32, tag="mt")
        hs49 = _bc(msk[:, 392:396].rearrange("n (s a) -> n s a", a=1),
                   [N, 4, 49])
        ws49 = _bc(msk[:, 396:400].rearrange("n (s a) -> n s a", a=1),
                   [N, 4, 49])
        nc.vector.tensor_tensor(out=m3(mt[:]), in0=bx49,
                                in1=m3(msk[:, 0:196]), op=OP.add)
        nc.vector.tensor_tensor(out=m3(mi[:]), in0=m3(mt[:]), in1=hs49,
                                op=OP.is_le)
        nc.vector.tensor_scalar(out=mt[:], in0=mt[:], scalar1=0,
                                scalar2=None, op0=OP.is_ge)
        nc.vector.tensor_tensor(out=mi[:], in0=mi[:], in1=mt[:],
                                op=OP.bitwise_and)
        nc.vector.tensor_tensor(out=m3(mt[:]), in0=by49,
                                in1=m3(msk[:, 196:392]), op=OP.add)
        mw = sp.tile([N, 196], I32, tag="mw")
        nc.vector.tensor_tensor(out=m3(mw[:]), in0=m3(mt[:]), in1=ws49,
                                op=OP.is_le)
        nc.vector.tensor_tensor(out=mi[:], in0=mi[:], in1=mw[:],
                                op=OP.bitwise_and)
        nc.vector.tensor_scalar(out=mt[:], in0=mt[:], scalar1=0,
                                scalar2=None, op0=OP.is_ge)
        nc.vector.tensor_tensor(out=mi[:], in0=mi[:], in1=mt[:],
                                op=OP.bitwise_and)
        mask = sp.tile([N, 196], F32, tag="mask")
        nc.vector.tensor_copy(out=mask[:], in_=mi[:])

        # ---- gathers + dots (mult+reduce split across DVE and Pool) ----
        dots = sp.tile([N, 196], F32, tag="dots")
        qf = sp.tile([N, CH], F32, tag="qf")
        qfb = sp.tile([N, CH], BF16, tag="qfb")
        run_order = [(0, 3), (0, 0), (0, 1), (0, 2), (0, 4), (0, 5), (0, 6)]
        run_order += [(s, dx) for s in range(1, 4) for dx in range(7)]
        for ri, (s, dx) in enumerate(run_order):
            K = gp.tile([N, 7 * CH], BF16, tag="K")
            nc.gpsimd.indirect_dma_start(
                out=K[:], out_offset=None, in_=maps_flat,
                in_offset=bass.IndirectOffsetOnAxis(
                    ap=idx[:, s * 7 + dx:s * 7 + dx + 1], axis=0))
            K3 = K[:].rearrange("n (d c) -> n d c", c=CH)
            if s == 0 and dx == 3:
                nc.vector.tensor_copy(out=qf[:], in_=K3[:, 3, :])
                nc.vector.tensor_copy(out=qfb[:], in_=K3[:, 3, :])
            eng = nc.gpsimd if ri % 3 == 2 else nc.vector
            Kf = kfp.tile([N, 7 * CH], BF16, tag="Kf")
            Kf3 = Kf[:].rearrange("n (d c) -> n d c", c=CH)
            eng.tensor_tensor(
                out=Kf3, in0=K3,
                in1=_bc(qfb[:].rearrange("n (a c) -> n a c", a=1),
                        [N, 7, CH]),
                op=OP.mult)
            j = s * 49 + dx * 7
            nc.vector.tensor_reduce(
                out=dots[:, j:j + 7].rearrange("n (d a) -> n d a", a=1),
                in_=Kf3, op=OP.add, axis=AX.X)
        nc.vector.tensor_tensor(out=dots[:], in0=dots[:], in1=mask[:],
                                op=OP.mult)

        # ---- AllGather (qf | dots), bf16 payload ----
        agi = sp.tile([N, CH + 196], BF16, tag="agi")
        nc.vector.tensor_copy(out=agi[:, 0:CH], in_=qfb[:])
        nc.vector.tensor_copy(out=agi[:, CH:], in_=dots[:])
        cin = ccp.tile([N, CH + 196], BF16, tag="cin")
        cout = ccp.tile([4 * N, CH + 196], BF16, tag="cout")
        nc.sync.dma_start(cin[:], agi[:])
        nc.gpsimd.collective_compute(
            "AllGather", OP.bypass, ins=[cin[:]], outs=[cout[:]],
            replica_groups=GROUPS)

        # ---- tokens ----
        for r in range(4):
            nc.sync.dma_start(tokb[:, CH * r:CH * (r + 1)],
                              cout[N * r:N * (r + 1), 0:CH])
        nc.vector.tensor_copy(out=tok[:, 0:1024], in_=tokb[:])
        dsum = sp.tile([N, 4, 196], BF16, tag="dsum")
        nc.sync.dma_start(
            dsum[:], cout[:].rearrange("(r n) e -> r n e", n=N)
            .rearrange("r n e -> n r e")[:, :, CH:])
        dsf = sp.tile([N, 2, 196], F32, tag="dsf")
        nc.vector.tensor_tensor(out=dsf[:, 0, :], in0=dsum[:, 0, :],
                                in1=dsum[:, 1, :], op=OP.add)
        nc.vector.tensor_tensor(out=dsf[:, 1, :], in0=dsum[:, 2, :],
                                in1=dsum[:, 3, :], op=OP.add)
        nc.vector.tensor_tensor(out=tok[:, 1024:1220], in0=dsf[:, 0, :],
                                in1=dsf[:, 1, :], op=OP.add)
        nc.vector.tensor_copy(out=tok[:, 1220:1222], in_=bnd[:])
        _ln(nc, sp, tok[:, 0:D], D, "l1")
        nc.vector.tensor_tensor(out=tok[:, 0:D], in0=tok[:, 0:D],
                                in1=pe_ap, op=OP.add)
        if it == 0:
            nc.sync.dma_start(dbg_tok[:], tok[:, 0:D])

        # ---- QKV: Q,K replicated full; V' 320-col slice ----
        xt = _tp_cols(nc, sp, pq, tok[:], DP, ident, "xt")
        qk_chains = [(0, 512, q_t, 0), (512, 512, q_t, 512),
                     (1024, 256, q_t, 1024),
                     (1280, 512, k_t, 0), (1792, 512, k_t, 512),
                     (2304, 256, k_t, 1024),
                     (2560, QS, v_t, 0)]
        for (c0, cw, dst, d0) in qk_chains:
            ps = pp.tile([N, 512], F32, tag="mmps", space="PSUM")
            for k in range(10):
                nc.tensor.matmul(ps[:, :cw], xt[:, k, :],
                                 w_qkv[:, k, c0:c0 + cw],
                                 start=(k == 0), stop=(k == 9))
            nc.vector.tensor_copy(out=dst[:, d0:d0 + cw], in_=ps[:, :cw])

        # ---- attention scores (replicated, no collective) ----
        qT = _tp_cols(nc, sp, pq, q_t[:], DP, identb, "qT")
        kT = _tp_cols(nc, sp, pq, k_t[:], DP, identb, "kT")
        sc_ps = pp.tile([N, N], F32, tag="mmps", space="PSUM")
        for k in range(10):
            nc.tensor.matmul(sc_ps[:], qT[:, k, :], kT[:, k, :],
                             start=(k == 0), stop=(k == 9))
        sc = sp.tile([N, N], F32, tag="sc")
        nc.vector.tensor_scalar(out=sc[:], in0=sc_ps[:], scalar1=inv_sqrt_d,
                                scalar2=None, op0=OP.mult)

        # ---- softmax (replicated) ----
        mx = sp.tile([N, 1], F32, tag="mx")
        nc.vector.tensor_reduce(out=mx[:], in_=sc[:], op=OP.max, axis=AX.X)
        nmx = sp.tile([N, 1], F32, tag="nmx")
        nc.vector.tensor_scalar(out=nmx[:], in0=mx[:], scalar1=-1.0,
                                scalar2=None, op0=OP.mult)
        esum = sp.tile([N, 1], F32, tag="esum")
        nc.scalar.activation(out=sc[:], in_=sc[:], func=AF.Exp,
                             bias=nmx[:], accum_out=esum[:])
        rsum = sp.tile([N, 1], F32, tag="rsum")
        nc.vector.reciprocal(out=rsum[:], in_=esum[:])
        nc.vector.tensor_scalar(out=sc[:], in0=sc[:], scalar1=rsum[:],
                                scalar2=None, op0=OP.mult)
        smT_ps = pq.tile([N, N], F32, tag="tpps", space="PSUM")
        nc.tensor.transpose(out=smT_ps[:], in_=sc[:], identity=ident[:N, :N])
        smTb = sp.tile([N, N], BF16, tag="smTb")
        nc.vector.tensor_copy(out=smTb[:], in_=smT_ps[:])

        # ---- attention output slice [N, 320] + AllGather (bf16) ----
        at_ps = pp.tile([N, QS], F32, tag="mmps", space="PSUM")
        nc.tensor.matmul(at_ps[:], smTb[:], v_t[:], start=True, stop=True)
        at_s = sp.tile([N, QS], BF16, tag="at_s")
        nc.vector.tensor_copy(out=at_s[:], in_=at_ps[:])
        ain = ccp.tile([N, QS], BF16, tag="ain")
        aout = ccp.tile([4 * N, QS], BF16, tag="aout")
        nc.sync.dma_start(ain[:], at_s[:])
        nc.gpsimd.collective_compute(
            "AllGather", OP.bypass, ins=[ain[:]], outs=[aout[:]],
            replica_groups=GROUPS)
        for r in range(4):
            nc.sync.dma_start(attb[:, QS * r:QS * (r + 1)],
                              aout[N * r:N * (r + 1), :])
        # x2 = attn + tok over full padded width (restores bias col 1222)
        nc.vector.tensor_copy(out=x2[:], in_=attb[:])
        nc.vector.tensor_tensor(out=x2[:], in0=x2[:], in1=tok[:], op=OP.add)
        _ln(nc, sp, x2[:, 0:D], D, "l2")

        # ---- FF: lin1 col-slice -> relu -> lin2 row-slice (partial) ----
        x2T = _tp_cols(nc, sp, pq, x2[:], DP, ident, "x2T")
        ps1 = pp.tile([N, FS], F32, tag="mmps", space="PSUM")
        for k in range(10):
            nc.tensor.matmul(ps1[:], x2T[:, k, :], w_l1[:, k, :],
                             start=(k == 0), stop=(k == 9))
        nc.vector.tensor_scalar(out=h[:, 0:FS], in0=ps1[:],
                                scalar1=0.0, scalar2=None, op0=OP.max)
        hT = _tp_cols(nc, sp, pq, h[:], FSP, identb, "hT")
        x3p = sp.tile([N, D], BF16, tag="x3p")
        for ccol in range(3):
            c0 = 512 * ccol
            cw = min(512, D - c0)
            ps = pp.tile([N, 512], F32, tag="mmps", space="PSUM")
            for k in range(5):
                nc.tensor.matmul(ps[:, :cw], hT[:, k, :],
                                 w_l2[:, k, c0:c0 + cw],
                                 start=(k == 0), stop=(k == 4))
            nc.vector.tensor_copy(out=x3p[:, c0:c0 + cw], in_=ps[:, :cw])
        xin = ccp.tile([N, D], BF16, tag="xin")
        xout = ccp.tile([4 * N, D], BF16, tag="xout")
        nc.sync.dma_start(xin[:], x3p[:])
        nc.gpsimd.collective_compute(
            "AllGather", OP.bypass, ins=[xin[:]], outs=[xout[:]],
            replica_groups=GROUPS)
        x3 = sp.tile([N, D], F32, tag="x3")
        xg = sp.tile([N, 2, D], BF16, tag="xg")
        xo4 = xout[:].rearrange("(r n) e -> r n e", n=N) \
            .rearrange("r n e -> n r e")
        nc.sync.dma_start(xg[:], xo4[:, 0:2, :])
        nc.vector.tensor_tensor(out=x3[:], in0=xg[:, 0, :],
                                in1=xg[:, 1, :], op=OP.add)
        nc.sync.dma_start(xg[:], xo4[:, 2:4, :])
        xf = sp.tile([N, D], F32, tag="lnsq")
        nc.vector.tensor_tensor(out=xf[:], in0=xg[:, 0, :],
                                in1=xg[:, 1, :], op=OP.add)
        nc.vector.tensor_tensor(out=x3[:], in0=x3[:], in1=xf[:], op=OP.add)
        nc.vector.tensor_tensor(out=x3[:], in0=x3[:], in1=x2[:, 0:D],
                                op=OP.add)
        _ln(nc, sp, x3[:], D, "l3")
        if it == 0:
            nc.sync.dma_start(dbg_x3[:], x3[:])

        # ---- fc head (only 2 outputs): DVE multiply, ACT accumulate ----
        f0 = sp.tile([N, D], F32, tag="lnsq")
        off = sp.tile([N, 2], F32, tag="off")
        nc.vector.tensor_tensor(out=f0[:], in0=x3[:], in1=fcw0, op=OP.mult)
        nc.scalar.activation(out=f0[:], in_=f0[:], func=AF.Copy,
                             accum_out=off[:, 0:1])
        f1 = sp.tile([N, D], F32, tag="f1")
        nc.vector.tensor_tensor(out=f1[:], in0=x3[:], in1=fcw1, op=OP.mult)
        nc.scalar.activation(out=f1[:], in_=f1[:], func=AF.Copy,
                             accum_out=off[:, 1:2])
        nc.vector.tensor_tensor(out=off[:], in0=off[:],
                                in1=cst[:, 3 * D:3 * D + 2], op=OP.add)
        if it == 0:
            nc.sync.dma_start(dbg_off[:], off[:])

        # trunc toward zero: rne(off - 0.5*sign(off)); exact ints unaffected
        sgn = sp.tile([N, 2], F32, tag="sgn")
        nc.scalar.activation(out=sgn[:], in_=off[:], func=AF.Sign)
        nc.vector.tensor_scalar(out=sgn[:], in0=sgn[:], scalar1=-0.5,
                                scalar2=None, op0=OP.mult)
        nc.vector.tensor_tensor(out=off[:], in0=off[:], in1=sgn[:],
                                op=OP.add)
        ti = sp.tile([N, 2], I32, tag="ti")
        nc.vector.tensor_copy(out=ti[:], in_=off[:])
        nc.vector.tensor_tensor(out=bnd[:], in0=bnd[:], in1=ti[:], op=OP.add)
        nc.vector.tensor_scalar(out=bnd[:], in0=bnd[:], scalar1=0,
                                scalar2=223, op0=OP.max, op1=OP.min)
        nc.sync.dma_start(traj[it, :, :], bnd[:])


# ---------------------------------------------------------------------------
# host side
# ---------------------------------------------------------------------------

_NC_CACHE = {}


def _host_inputs(curr_img_features, previous_boundary, in_proj_w, in_proj_b,
                 out_proj_w, out_proj_b, lin1_w, lin1_b, lin2_w, lin2_b,
                 fc_w, fc_b):
    import ml_dtypes
    f32 = np.float32
    bf = ml_dtypes.bfloat16
    pos = np.arange(N, dtype=f32)[:, None]
    div = np.exp(np.arange(0, D, 2, dtype=f32) * (-np.log(10000.0) / D))
    pe = np.zeros((N, D), f32)
    pe[:, 0::2] = np.sin(pos * div)
    pe[:, 1::2] = np.cos(pos * div)

    Wq, Wk, Wv = (np.asarray(in_proj_w[i * D:(i + 1) * D], f32)
                  for i in range(3))
    bq, bk, bv = (np.asarray(in_proj_b[i * D:(i + 1) * D], f32)
                  for i in range(3))
    Wvp = np.asarray(out_proj_w, f32) @ Wv          # [D, D]
    bvp = np.asarray(out_proj_w, f32) @ bv + np.asarray(out_proj_b, f32)

    # padded [DP, 3*DP]: rows = input dim (row D = bias)
    qkv_full = np.zeros((DP, 3 * DP), f32)
    for i, (W, b) in enumerate(((Wq, bq), (Wk, bk), (Wvp, bvp))):
        qkv_full[0:D, DP * i:DP * i + D] = W.T
        qkv_full[D, DP * i:DP * i + D] = b

    l1 = np.zeros((DP, FF), f32)
    l1[0:D, :] = np.asarray(lin1_w, f32).T
    l1[D, :] = np.asarray(lin1_b, f32)
    l2t = np.asarray(lin2_w, f32).T                  # [FF, D]
    l2b = np.asarray(lin2_b, f32)

    cst = np.zeros((N, 3 * D + 2), f32)
    cst[:, 0:D] = pe
    cst[:, D:2 * D] = np.asarray(fc_w[:, 0, :], f32)
    cst[:, 2 * D:3 * D] = np.asarray(fc_w[:, 1, :], f32)
    cst[:, 3 * D:3 * D + 2] = np.asarray(fc_b[:, :2], f32)

    tbl = np.zeros((168,), np.int32)
    for s in range(4):
        Hs, Ws = SCALE_HW[s]
        for dx in range(7):
            j = s * 7 + dx
            tbl[j] = dx - 3
            tbl[28 + j] = Hs - 1
            tbl[56 + j] = Ws
            tbl[84 + j] = SCALE_BASE[s]
            tbl[112 + j] = Hs * Ws - 7
    tbl[140:144] = [0, 1, 2, 3]
    tblr = np.tile(tbl[None, :], (N, 1))

    mskv = np.zeros((400,), np.int32)
    for s in range(4):
        Hs, Ws = SCALE_HW[s]
        for dx in range(7):
            for dy in range(7):
                j = s * 49 + dx * 7 + dy
                mskv[j] = dx - 3
                mskv[196 + j] = dy - 3
        mskv[392 + s] = Hs - 1
        mskv[396 + s] = Ws - 1
    mskr = np.tile(mskv[None, :], (N, 1))

    ident = np.eye(P, dtype=f32)

    imgs = np.asarray(curr_img_features, f32)
    bnds = np.asarray(previous_boundary, np.int32)
    in_maps = []
    for c in range(8):
        g, q = c // 4, c % 4
        # Q full | K full | V' col slice for this core
        qs = np.ascontiguousarray(np.concatenate(
            [qkv_full[:, 0:DP], qkv_full[:, DP:2 * DP],
             qkv_full[:, 2 * DP + QS * q:2 * DP + QS * (q + 1)]],
            axis=1)).astype(bf)                      # [DP, 2880]
        l1s = np.ascontiguousarray(
            l1[:, FS * q:FS * (q + 1)]).astype(bf)   # [DP, 512]
        l2s = np.zeros((FSP, D), f32)
        l2s[0:FS, :] = l2t[FS * q:FS * (q + 1), :]
        if q == 0:
            l2s[FS, :] = l2b                         # bias row (once)
        m = dict(tbl_in=tblr, msk_in=mskr, cst_in=cst, ident_in=ident,
                 qkvw=qs, lin1w=l1s, lin2w=l2s.astype(bf))
        m["img"] = np.ascontiguousarray(imgs[g, CH * q:CH * (q + 1)])
        m["bnd_in"] = np.ascontiguousarray(bnds[g])
        in_maps.append(m)
    return in_maps


def make_in_maps(inputs):
    return _host_inputs(
        inputs["curr_img_features"], inputs["previous_boundary"],
        inputs["in_proj_w"], inputs["in_proj_b"],
        inputs["out_proj_w"], inputs["out_proj_b"],
        inputs["lin1_w"], inputs["lin1_b"],
        inputs["lin2_w"], inputs["lin2_b"],
        inputs["fc_w"], inputs["fc_b"])


def kernel(**inputs):
    from concourse.bass_utils import run_bass_kernel_spmd
    install_profile_hook()

    in_maps = make_in_maps(inputs)
    if "nc" not in _NC_CACHE:
        _NC_CACHE["nc"] = build_kernel()
    nc = _NC_CACHE["nc"]
    res = run_bass_kernel_spmd(nc, in_maps, core_ids=list(range(8)))
    kernel.last_results = res
    t0 = res.results[0]["traj"]   # batch 0
    t1 = res.results[4]["traj"]   # batch 1
    return np.stack([t0, t1], axis=1).astype(np.int32)  # [6, 2, 80, 2]
